# revision 42
# baseline (speedup 1.0000x reference)
"""GNN NodeBlock (message passing + 3-layer MLP + LayerNorm) on 8 Trainium2 cores.

Strategy (data parallel over nodes):
  - Shard 50000 nodes across 8 cores (6250 each, padded to 6272 = 49*128).
  - Segment-sum of edge features runs on the *Vector engine*, not TensorE:
    the host lays edges out degree-layered as eflay[tile, 96f, 128n, D1]
    (layer d = d-th incoming edge of each node, zero-padded), which the DVE
    reduces with two bf16 halving adds (2x mode) + one fp32 tensor_reduce.
    Nodes with degree > D1 spill into a tiny per-tile one-hot matmul tail
    (capacity CT chunks of 128 edges) on TensorE, combined during the cast
    to bf16. This frees ~55us of TensorE time vs an all-matmul segment sum.
  - The MLP runs entirely in T-layout (features on partitions, nodes on the
    free dim) with weights stationary: h^T = W.T @ x^T, so no transposes are
    needed between layers. Node features enter pre-transposed from the host.
  - Layer 3 swaps the operands (activations stationary) to produce y in natural
    layout [128 nodes, 512 feats]; bias b3 is added on VectorE. LayerNorm
    reduces over the free dim: bn_stats/bn_aggr (VectorE) + sqrt (ScalarE) +
    reciprocal (VectorE), applied via one ScalarE activation with
    per-partition scale/bias.
  - The lone 49th tile runs as the FIRST group (small working set => early
    TensorE start) so the drain ends on a dense 4-tile group.
  - All matmuls are bf16 inputs with fp32 PSUM accumulation (~4e-3 L2 rel err).

Everything is compiled once per (CT, apply_gamma_beta) configuration and cached.
"""

import numpy as np
import ml_dtypes

P = 128
NODE_DIM = 512
EDGE_DIM = 96
HID = 1024
OUT = 512
N_NODES = 50000
N_EDGES = 800000
NCORES = 8
LN_EPS = 1e-5

NPC = N_NODES // NCORES          # 6250 nodes per core
T_TILES = -(-NPC // P)           # 49 node tiles per core
NPAD = T_TILES * P               # 6272
GMAX = 4                         # node tiles per super-tile (NT = 512 free dim)
D1 = 20                          # degree layers summed on the Vector engine

BF16 = ml_dtypes.bfloat16

_CACHE: dict = {}


# ----------------------------------------------------------------------------
# Bass program
# ----------------------------------------------------------------------------

def _build_program(ct: int, apply_gamma_beta: bool):
    import concourse.bass as bass
    import concourse.bacc as bacc
    import concourse.mybir as mybir
    import concourse.tile as tile

    f32 = mybir.dt.float32
    bf16 = mybir.dt.bfloat16
    Act = mybir.ActivationFunctionType
    Alu = mybir.AluOpType
    Ax = mybir.AxisListType

    KD = NODE_DIM // P           # 4 node-feat k-chunks
    KH = HID // P                # 8 hidden k-chunks
    MH = HID // P                # 8 hidden m-chunks
    KD1 = KD + 1                 # + 1 chunk for the 96 agg features
    D2 = D1 // 2                 # 10
    D4 = D1 // 4                 # 5

    nc = bacc.Bacc("TRN2", target_bir_lowering=False, debug=False)

    # inputs (per core)
    eflay_d = nc.declare_dram_parameter("eflay", [T_TILES, EDGE_DIM, P, D1], bf16, isOutput=False)
    eftl_d = nc.declare_dram_parameter("eftl", [P, T_TILES * ct * P], bf16, isOutput=False)
    dstl_d = nc.declare_dram_parameter("dstl", [P, T_TILES, ct], bf16, isOutput=False)
    # node features blocked [partition, tile, k, node-in-tile]: a group's DMA
    # is one 4KB-per-partition contiguous read (descriptor-efficient)
    nfg_d = nc.declare_dram_parameter("nfg", [P, T_TILES, KD, P], bf16, isOutput=False)
    w1_d = nc.declare_dram_parameter("w1", [P, KD1 * MH * P], bf16, isOutput=False)
    w2_d = nc.declare_dram_parameter("w2", [P, KH * MH * P], bf16, isOutput=False)
    w3_d = nc.declare_dram_parameter("w3", [P, KH * OUT], bf16, isOutput=False)
    # cstB: b1T(MH) | b2T(MH); cstLN: gamma(OUT) | beta(OUT) | b3(OUT) | eps(1)
    cstB_d = nc.declare_dram_parameter("cstB", [P, 2 * MH], f32, isOutput=False)
    cstLN_d = nc.declare_dram_parameter("cstLN", [P, 3 * OUT + 1], f32, isOutput=False)
    cstb_d = nc.declare_dram_parameter("cstb", [P, P], bf16, isOutput=False)
    y_d = nc.declare_dram_parameter("y", [NPAD, OUT], f32, isOutput=True)

    # tile 48 (the ragged one) runs first, then group sizes ramp 2, 3, 4...:
    # the start of the kernel is HBM-bound, so early groups keep their input
    # working set small enough to arrive in time.
    groups = [(T_TILES - 1, 1)]
    t0 = 0
    ramp = [2, 3]
    while t0 < T_TILES - 1:
        g = min(ramp.pop(0) if ramp else GMAX, T_TILES - 1 - t0)
        groups.append((t0, g))
        t0 += g

    with tile.TileContext(nc) as tc:
        with (
            tc.tile_pool(name="const", bufs=1) as constp,
            tc.tile_pool(name="lay", bufs=6) as layp,
            tc.tile_pool(name="lay2", bufs=4) as lay2p,
            tc.tile_pool(name="lay3", bufs=4) as lay3p,
            tc.tile_pool(name="ef", bufs=6) as efp,
            tc.tile_pool(name="oh", bufs=6) as ohp,
            tc.tile_pool(name="ag32", bufs=2) as ag32p,
            tc.tile_pool(name="agg", bufs=3) as aggp,
            tc.tile_pool(name="nfx", bufs=3) as nfxp,
            tc.tile_pool(name="h1", bufs=2) as h1p,
            tc.tile_pool(name="h2", bufs=2) as h2p,
            tc.tile_pool(name="yo", bufs=3) as yop,
            tc.tile_pool(name="st", bufs=8) as stp,
            tc.tile_pool(name="psA", bufs=2, space="PSUM") as psA,
            tc.tile_pool(name="psM", bufs=3, space="PSUM") as psM,
            tc.tile_pool(name="psY", bufs=3, space="PSUM") as psY,
        ):
            # small constants first so the tail path of group 0 clears quickly
            dstl_sb = constp.tile([P, T_TILES, ct], bf16)
            nc.sync.dma_start(out=dstl_sb[:], in_=dstl_d[:, :, :])
            cstb_sb = constp.tile([P, P], bf16)
            nc.sync.dma_start(out=cstb_sb[:], in_=cstb_d[:, :])
            cstB_sb = constp.tile([P, 2 * MH], f32)
            nc.sync.dma_start(out=cstB_sb[:], in_=cstB_d[:, :])
            # group 0's slice of the tail-edge table as its own tiny tile so
            # its one-hot matmuls never wait on the full-table load
            t00 = groups[0][0]
            eftl0_sb = constp.tile([P, ct * P], bf16)
            nc.sync.dma_start(out=eftl0_sb[:],
                              in_=eftl_d[:, t00 * ct * P:(t00 + 1) * ct * P])
            w1_sb = constp.tile([P, MH * KD1 * P], bf16)
            w2_sb = constp.tile([P, MH * KH * P], bf16)
            w3_sb = constp.tile([P, KH * OUT], bf16)
            cstLN_sb = constp.tile([P, 3 * OUT + 1], f32)
            # whole tail-edge table stays resident (12.5KB/partition); group
            # 1's slice loads at startup, the remainder during group 1's body
            eftl_sb = constp.tile([P, T_TILES * ct * P], bf16)
            t10, g1n = groups[1]
            nc.sync.dma_start(
                out=eftl_sb[:, t10 * ct * P:(t10 + g1n) * ct * P],
                in_=eftl_d[:, t10 * ct * P:(t10 + g1n) * ct * P])

            def emit_agg_dmas(tstart, g):
                """Issue the edge-data DMAs for a group one group ahead of
                their compute so the reduces never wait on them."""
                lays = []
                for s in range(g):
                    lay = layp.tile([EDGE_DIM, P, D1], bf16, tag="lay", name="lay")
                    nc.sync.dma_start(out=lay[:], in_=eflay_d[tstart + s, :, :, :])
                    lays.append(lay)
                return lays

            def alloc_agg():
                aggT = aggp.tile([EDGE_DIM, GMAX * P], bf16, tag="agg",
                                 name="aggT")
                agg32 = ag32p.tile([EDGE_DIM, GMAX * P], f32, tag="ag32",
                                   name="agg32")
                ps_t = psA.tile([P, GMAX * P], f32, tag="psA")
                return aggT, agg32, ps_t

            def emit_onehot(t):
                """One-hot of a tile's tail-edge destinations; emitted well
                ahead of the tail matmul so the PE FIFO never waits on it."""
                oh_t = ohp.tile([P, ct, P], bf16, tag="oh", name="oh_t")
                nc.vector.tensor_tensor(
                    out=oh_t[:],
                    in0=dstl_sb[:, t, :, None].to_broadcast([P, ct, P]),
                    in1=cstb_sb[:, None, 0:P].to_broadcast([P, ct, P]),
                    op=Alu.is_equal,
                )
                return oh_t

            def emit_agg_chain(t, s, lay, oh_t, aggT, agg32, ps_t, ef_src=None):
                """Segment-sum of node tile t into aggT[:, s*P:(s+1)*P].

                Vector engine does the bulk (degree-layered reduce); TensorE
                adds the high-degree tail via ct one-hot chunk matmuls into a
                PSUM bank shared by the whole group (no cross-subtile PSUM
                dependency)."""
                for j in range(ct):
                    if ef_src is None:
                        lhsT = eftl_sb[:, (t * ct + j) * P:(t * ct + j + 1) * P]
                    else:
                        lhsT = ef_src[:, j * P:(j + 1) * P]
                    nc.tensor.matmul(
                        out=ps_t[:, s * P:(s + 1) * P],
                        lhsT=lhsT,
                        rhs=oh_t[:, j, :],
                        start=(j == 0),
                        stop=(j == ct - 1),
                    )
                lay2 = lay2p.tile([EDGE_DIM, P, D2], bf16, tag="lay2")
                nc.vector.tensor_tensor(
                    out=lay2[:], in0=lay[:, :, 0:D2], in1=lay[:, :, D2:D1],
                    op=Alu.add,
                )
                lay3 = lay3p.tile([EDGE_DIM, P, D4], bf16, tag="lay3")
                nc.vector.tensor_tensor(
                    out=lay3[:], in0=lay2[:, :, 0:D4], in1=lay2[:, :, D4:D2],
                    op=Alu.add,
                )
                nc.vector.tensor_reduce(
                    out=agg32[:, s * P:(s + 1) * P],
                    in_=lay3[:], axis=Ax.X, op=Alu.add,
                )
                # combine tail + cast to bf16 for the L1 matmul rhs
                nc.vector.tensor_tensor(
                    out=aggT[:, s * P:(s + 1) * P],
                    in0=agg32[:, s * P:(s + 1) * P],
                    in1=ps_t[0:EDGE_DIM, s * P:(s + 1) * P],
                    op=Alu.add,
                )

            def emit_nfx_dma(tstart, g):
                nfx = nfxp.tile([P, GMAX, KD, P], bf16, tag="nfx")
                nc.sync.dma_start(out=nfx[:, 0:g, :, :],
                                  in_=nfg_d[:, tstart:tstart + g, :, :])
                return nfx

            # group 0's aggregation up front (weight DMAs stream in behind it)
            agg_tiles = {}
            agg_dmas = {}
            nfx_tiles = {}
            agg_dmas[0] = emit_agg_dmas(*groups[0])
            nfx_tiles[0] = emit_nfx_dma(*groups[0])
            t00_, g00 = groups[0]
            agg_tiles[0] = alloc_agg()
            for s in range(g00):
                emit_agg_chain(t00_ + s, s, agg_dmas[0][s],
                               emit_onehot(t00_ + s), *agg_tiles[0],
                               ef_src=eftl0_sb)
            agg_dmas.pop(0)

            for gi, (tstart, g) in enumerate(groups):
                nt = g * P  # free-dim width of this super-tile
                n0 = tstart * P
                aggT, _, _ = agg_tiles.pop(gi)
                nfx = nfx_tiles.pop(gi)

                # next group's input DMAs go out a full group early (except
                # behind group 0's weight DMAs: the ramp is HBM-bound and L1
                # needs w1 slices before any of group 1's edge data)
                if gi == 1:
                    # rest of the tail-edge table (tiles outside groups 0/1)
                    if t10 > 0:
                        nc.sync.dma_start(out=eftl_sb[:, 0:t10 * ct * P],
                                          in_=eftl_d[:, 0:t10 * ct * P])
                    if (t10 + g1n) < t00:
                        nc.sync.dma_start(
                            out=eftl_sb[:, (t10 + g1n) * ct * P:t00 * ct * P],
                            in_=eftl_d[:, (t10 + g1n) * ct * P:t00 * ct * P])
                if gi + 1 < len(groups) and gi > 0:
                    agg_dmas[gi + 1] = emit_agg_dmas(*groups[gi + 1])
                    nfx_tiles[gi + 1] = emit_nfx_dma(*groups[gi + 1])
                if gi == 0:
                    # per-m weight slices stream in behind group 0's agg work so
                    # layer 1/2 can begin as soon as their own slice lands
                    for m in range(MH):
                        nc.sync.dma_start(
                            out=w1_sb[:, m * KD1 * P:(m + 1) * KD1 * P],
                            in_=w1_d[:, m * KD1 * P:(m + 1) * KD1 * P])

                # ---- layer 1: h1T[m] = relu(W1.T @ xT + b1), x = [nf; agg] ----
                h1 = h1p.tile([P, KH, GMAX * P], bf16, tag="h1")
                for m in range(MH):
                    ps = psM.tile([P, GMAX * P], f32, tag="psM")
                    for k in range(KD):
                        nc.tensor.matmul(
                            out=ps[:, 0:nt],
                            lhsT=w1_sb[:, (m * KD1 + k) * P:(m * KD1 + k + 1) * P],
                            rhs=nfx[:, 0:g, k, :],
                            start=(k == 0),
                            stop=False,
                        )
                    nc.tensor.matmul(
                        out=ps[:, 0:nt],
                        lhsT=w1_sb[0:EDGE_DIM, (m * KD1 + KD) * P:(m * KD1 + KD) * P + P],
                        rhs=aggT[:, 0:nt],
                        start=False,
                        stop=True,
                    )
                    nc.scalar.activation(
                        out=h1[:, m, 0:nt], in_=ps[:, 0:nt], func=Act.Relu,
                        bias=cstB_sb[:, m:m + 1],
                    )
                    if gi == 0:
                        nc.sync.dma_start(
                            out=w2_sb[:, m * KH * P:(m + 1) * KH * P],
                            in_=w2_d[:, m * KH * P:(m + 1) * KH * P])
                        # interleave group 1's edge/node data between the w2
                        # slices: its reduces start during this group's L2,
                        # well before w2's tail is needed
                        if m == 1 and len(groups) > 1:
                            agg_dmas[1] = emit_agg_dmas(*groups[1])
                        if m == 3 and len(groups) > 1:
                            nfx_tiles[1] = emit_nfx_dma(*groups[1])

                # ---- layer 2 ----
                h2 = h2p.tile([P, KH, GMAX * P], bf16, tag="h2")
                for m in range(MH):
                    ps = psM.tile([P, GMAX * P], f32, tag="psM")
                    for k in range(KH):
                        nc.tensor.matmul(
                            out=ps[:, 0:nt],
                            lhsT=w2_sb[:, (m * KH + k) * P:(m * KH + k + 1) * P],
                            rhs=h1[:, k, 0:nt],
                            start=(k == 0),
                            stop=(k == KH - 1),
                        )
                    nc.scalar.activation(
                        out=h2[:, m, 0:nt], in_=ps[:, 0:nt], func=Act.Relu,
                        bias=cstB_sb[:, MH + m:MH + m + 1],
                    )
                    if gi == 0 and m < 2:
                        if m == 0:
                            nc.sync.dma_start(out=w3_sb[:], in_=w3_d[:, :])
                        else:
                            nc.sync.dma_start(out=cstLN_sb[:], in_=cstLN_d[:, :])

                # ---- layer 3 (nodes on partitions) + LayerNorm ----
                # the next group's aggregation interleaves per subtile, each
                # chain emitted AFTER that subtile's LN ops so the Vector FIFO
                # never delays the LN chain (psY slack absorbs the rest)
                if gi + 1 < len(groups):
                    tstart_nx, g_nx = groups[gi + 1]
                    agg_tiles[gi + 1] = alloc_agg()
                    lays_nx = agg_dmas.pop(gi + 1)
                    ohs_nx = [emit_onehot(tstart_nx + s) for s in range(g_nx)]
                else:
                    tstart_nx, g_nx = 0, 0
                for s in range(max(g, g_nx)):
                    if s >= g:
                        emit_agg_chain(tstart_nx + s, s, lays_nx[s],
                                       ohs_nx[s], *agg_tiles[gi + 1])
                        continue
                    ps_y = psY.tile([P, OUT], f32, tag="psY")
                    for k in range(KH):
                        nc.tensor.matmul(
                            out=ps_y[:],
                            lhsT=h2[:, k, s * P:(s + 1) * P],
                            rhs=w3_sb[:, k * OUT:(k + 1) * OUT],
                            start=(k == 0),
                            stop=(k == KH - 1),
                        )
                    # + b3 (broadcast rows) on VectorE, off the TensorE critical path
                    nc.vector.tensor_tensor(
                        out=ps_y[:], in0=ps_y[:],
                        in1=cstLN_sb[:, 2 * OUT:3 * OUT],
                        op=Alu.add,
                    )
                    st6 = stp.tile([P, 6], f32, tag="st6")
                    nc.vector.bn_stats(st6[:], ps_y[:])
                    mv = stp.tile([P, 2], f32, tag="mv")
                    nc.vector.bn_aggr(mv[:], st6[:])
                    std = stp.tile([P, 1], f32, tag="std")
                    nc.scalar.activation(std[:], mv[:, 1:2], Act.Sqrt,
                                         bias=cstLN_sb[:, 3 * OUT:])
                    rstd = stp.tile([P, 1], f32, tag="rstd")
                    nc.vector.reciprocal(rstd[:], std[:])
                    nmr = stp.tile([P, 1], f32, tag="nmr")
                    nc.vector.tensor_scalar(
                        out=nmr[:], in0=mv[:, 0:1], scalar1=rstd[:], scalar2=-1.0,
                        op0=Alu.mult, op1=Alu.mult,
                    )
                    yn = yop.tile([P, OUT], f32, tag="yn")
                    nc.scalar.activation(
                        out=yn[:], in_=ps_y[:], func=Act.Identity,
                        bias=nmr[:], scale=rstd[:],
                    )
                    if apply_gamma_beta:
                        nc.vector.tensor_tensor(
                            out=yn[:], in0=yn[:],
                            in1=cstLN_sb[:, 0:OUT], op=Alu.mult,
                        )
                        nc.vector.tensor_tensor(
                            out=yn[:], in0=yn[:],
                            in1=cstLN_sb[:, OUT:2 * OUT], op=Alu.add,
                        )
                    r0 = (tstart + s) * P
                    nc.sync.dma_start(out=y_d[r0:r0 + P, :], in_=yn[:])
                    if s < g_nx:
                        emit_agg_chain(tstart_nx + s, s, lays_nx[s],
                                       ohs_nx[s], *agg_tiles[gi + 1])

    nc.compile()
    return nc


# ----------------------------------------------------------------------------
# Host-side sharding / layout prep
# ----------------------------------------------------------------------------

def _prep_core(c, node_feat, edge_feat, dst, ct):
    KD_ = NODE_DIM // P
    lo = c * NPC
    sel = np.flatnonzero((dst >= lo) & (dst < lo + NPC))
    d = (dst[sel] - lo).astype(np.int64)
    order = np.argsort(d, kind="stable")
    sel = sel[order]
    d = d[order]
    counts = np.bincount(d, minlength=NPAD)
    offs = np.zeros(NPAD, np.int64)
    np.cumsum(counts[:-1], out=offs[1:])
    rank = np.arange(d.size) - offs[d]

    # main: first D1 edges of each node, degree-layered [T, 96, 128, D1]
    main = rank < D1
    flat = np.zeros((NPAD * D1, EDGE_DIM), np.float32)
    flat[d[main] * D1 + rank[main]] = edge_feat[sel[main]]
    eflay = np.ascontiguousarray(
        flat.astype(BF16).reshape(T_TILES, P, D1, EDGE_DIM).transpose(0, 3, 1, 2))

    # tail: edges beyond D1 per node, chunked one-hot layout per tile
    tail = np.flatnonzero(rank >= D1)
    dt_ = d[tail]
    tile_of = dt_ >> 7
    tcounts = np.bincount(tile_of, minlength=T_TILES)
    toffs = np.zeros(T_TILES, np.int64)
    np.cumsum(tcounts[:-1], out=toffs[1:])
    trank = np.arange(dt_.size) - toffs[tile_of]
    p_slot = trank % P
    c_slot = trank // P
    assert c_slot.max(initial=0) < ct

    eftl = np.zeros((T_TILES, P, ct, P), BF16)
    eftl[tile_of, p_slot, c_slot, :EDGE_DIM] = edge_feat[sel[tail]].astype(BF16)
    # resident layout: [partition(edge slot), tile*chunk*feat]
    eftl = np.ascontiguousarray(eftl.transpose(1, 0, 2, 3)).reshape(P, -1)
    dstl = np.full((T_TILES, P, ct), -1.0, BF16)
    dstl[tile_of, p_slot, c_slot] = (dt_ - (tile_of << 7)).astype(BF16)
    dstl = np.ascontiguousarray(dstl.transpose(1, 0, 2))

    # node features blocked [partition, tile, k, node-in-tile]:
    # nfg[p, t, k, j] = node_feat[t*128+j, k*128+p]
    nfp = np.zeros((NPAD, NODE_DIM), np.float32)
    nfp[:NPC] = node_feat[lo:lo + NPC]
    nfg = np.ascontiguousarray(
        nfp.astype(BF16).reshape(T_TILES, P, KD_, P).transpose(3, 0, 2, 1))
    return {"eflay": eflay, "eftl": eftl, "dstl": dstl, "nfg": nfg}


def _prep_shared(W1, b1, W2, b2, W3, b3, gamma, beta):
    KD1 = NODE_DIM // P + 1
    MH = HID // P
    KH = HID // P

    w1p = np.zeros((KD1 * P, HID), np.float32)
    w1p[:NODE_DIM + EDGE_DIM] = W1
    # m-major: col index (m*KD1 + k)*P + j
    w1 = np.ascontiguousarray(
        w1p.reshape(KD1, P, MH, P).transpose(1, 2, 0, 3)).reshape(P, -1).astype(BF16)
    w2 = np.ascontiguousarray(
        W2.reshape(KH, P, MH, P).transpose(1, 2, 0, 3)).reshape(P, -1).astype(BF16)
    w3 = np.ascontiguousarray(
        W3.reshape(KH, P, OUT).transpose(1, 0, 2)).reshape(P, -1).astype(BF16)

    cstB = np.ascontiguousarray(np.concatenate(
        [b1.reshape(MH, P).T, b2.reshape(MH, P).T], axis=1).astype(np.float32))
    cstLN = np.ascontiguousarray(np.concatenate([
        np.tile(gamma.reshape(1, OUT), (P, 1)),
        np.tile(beta.reshape(1, OUT), (P, 1)),
        np.tile(b3.reshape(1, OUT), (P, 1)),
        np.full((P, 1), LN_EPS, np.float32),
    ], axis=1).astype(np.float32))

    cstb = np.tile(np.arange(P, dtype=np.float32)[None, :], (P, 1)).astype(BF16)
    return {"w1": w1, "w2": w2, "w3": w3, "cstB": cstB, "cstLN": cstLN, "cstb": cstb}


# ----------------------------------------------------------------------------
# Entry point
# ----------------------------------------------------------------------------

def _ensure_axon_hooks_importable():
    """bass_utils imports antenv.axon_hooks when tracing is requested (even via
    the BASS_TRACE env var); provide a no-op stub if the module is absent so
    that path degrades to trace-skipped instead of crashing."""
    try:
        import antenv.axon_hooks  # noqa: F401
    except Exception:
        import sys
        import types
        try:
            import antenv
        except Exception:
            return
        mod = types.ModuleType('antenv.axon_hooks')
        mod._hook = None
        mod.set_axon_ntff_profile_hook = lambda h: setattr(mod, '_hook', h)
        mod.get_axon_ntff_profile_hook = lambda: mod._hook
        sys.modules['antenv.axon_hooks'] = mod
        antenv.axon_hooks = mod


def kernel(node_feat, edge_feat, edge_index, n_nodes, W1, b1, W2, b2, W3, b3,
           gamma, beta, _want_trace=False):
    from concourse.bass_utils import run_bass_kernel_spmd
    _ensure_axon_hooks_importable()

    node_feat = np.asarray(node_feat, dtype=np.float32)
    edge_feat = np.asarray(edge_feat, dtype=np.float32)
    edge_index = np.asarray(edge_index)
    assert int(n_nodes) == N_NODES
    assert node_feat.shape == (N_NODES, NODE_DIM)
    assert edge_feat.shape == (N_EDGES, EDGE_DIM)

    dst = edge_index[1].astype(np.int64)

    # tail capacity: chunks of 128 edges per tile beyond D1 per node (global,
    # so the SPMD program is shared by all cores)
    counts = np.bincount(dst, minlength=N_NODES)
    padded = np.zeros((NCORES, NPAD), np.int64)
    padded[:, :NPC] = counts.reshape(NCORES, NPC)
    tail_tile = np.maximum(padded - D1, 0).reshape(NCORES, T_TILES, P).sum(axis=2)
    ct = max(1, -(-int(tail_tile.max()) // P))

    gamma = np.asarray(gamma, dtype=np.float32)
    beta = np.asarray(beta, dtype=np.float32)
    apply_gb = not (np.all(gamma == 1.0) and np.all(beta == 0.0))

    key = (ct, apply_gb)
    if key not in _CACHE:
        _CACHE[key] = _build_program(ct, apply_gb)
    nc = _CACHE[key]

    shared = _prep_shared(
        np.asarray(W1, np.float32), np.asarray(b1, np.float32),
        np.asarray(W2, np.float32), np.asarray(b2, np.float32),
        np.asarray(W3, np.float32), np.asarray(b3, np.float32),
        gamma, beta)

    in_maps = []
    for c in range(NCORES):
        m = _prep_core(c, node_feat, edge_feat, dst, ct)
        m.update(shared)
        in_maps.append(m)

    res = run_bass_kernel_spmd(nc, in_maps, list(range(NCORES)), trace=_want_trace)

    y = np.concatenate([res.results[c]["y"][:NPC] for c in range(NCORES)], axis=0)
    out = np.ascontiguousarray(y, dtype=np.float32)
    if _want_trace:
        kernel.last_results = res
    return out


kernel.last_results = None


# revision 43
# speedup vs baseline: 1.1714x; 1.1714x over previous
"""GNN NodeBlock (message passing + 3-layer MLP + LayerNorm) on 8 Trainium2 cores.

Strategy (data parallel over nodes):
  - Shard 50000 nodes across 8 cores (6250 each, padded to 6272 = 49*128).
  - Segment-sum of edge features runs on the *Vector engine*, not TensorE:
    the host lays edges out degree-layered as eflay[tile, 96f, 128n, D1]
    (layer d = d-th incoming edge of each node, zero-padded), which the DVE
    reduces with two bf16 halving adds (2x mode) + one fp32 tensor_reduce.
    Nodes with degree > D1 spill into a tiny per-tile one-hot matmul tail
    (capacity CT chunks of 128 edges) on TensorE, combined during the cast
    to bf16. This frees ~55us of TensorE time vs an all-matmul segment sum.
  - The MLP runs entirely in T-layout (features on partitions, nodes on the
    free dim) with weights stationary: h^T = W.T @ x^T, so no transposes are
    needed between layers. Node features enter pre-transposed from the host.
  - Layer 3 swaps the operands (activations stationary) to produce y in natural
    layout [128 nodes, 512 feats]; bias b3 is added on VectorE. LayerNorm
    reduces over the free dim: bn_stats/bn_aggr (VectorE) + sqrt (ScalarE) +
    reciprocal (VectorE), applied via one ScalarE activation with
    per-partition scale/bias.
  - The lone 49th tile runs as the FIRST group (small working set => early
    TensorE start) so the drain ends on a dense 4-tile group.
  - All matmuls are bf16 inputs with fp32 PSUM accumulation (~4e-3 L2 rel err).

Everything is compiled once per (CT, apply_gamma_beta) configuration and cached.
"""

import numpy as np
import ml_dtypes

P = 128
NODE_DIM = 512
EDGE_DIM = 96
HID = 1024
OUT = 512
N_NODES = 50000
N_EDGES = 800000
NCORES = 8
LN_EPS = 1e-5

NPC = N_NODES // NCORES          # 6250 nodes per core
T_TILES = -(-NPC // P)           # 49 node tiles per core
NPAD = T_TILES * P               # 6272
GMAX = 4                         # node tiles per super-tile (NT = 512 free dim)
D1 = 20                          # degree layers summed on the Vector engine

BF16 = ml_dtypes.bfloat16

_CACHE: dict = {}


# ----------------------------------------------------------------------------
# Bass program
# ----------------------------------------------------------------------------

def _build_program(ct: int, apply_gamma_beta: bool):
    import concourse.bass as bass
    import concourse.bacc as bacc
    import concourse.mybir as mybir
    import concourse.tile as tile

    f32 = mybir.dt.float32
    bf16 = mybir.dt.bfloat16
    Act = mybir.ActivationFunctionType
    Alu = mybir.AluOpType
    Ax = mybir.AxisListType

    KD = NODE_DIM // P           # 4 node-feat k-chunks
    KH = HID // P                # 8 hidden k-chunks
    MH = HID // P                # 8 hidden m-chunks
    KD1 = KD + 1                 # + 1 chunk for the 96 agg features
    D2 = D1 // 2                 # 10
    D4 = D1 // 4                 # 5

    nc = bacc.Bacc("TRN2", target_bir_lowering=False, debug=False)

    # inputs (per core)
    eflay_d = nc.declare_dram_parameter("eflay", [T_TILES, EDGE_DIM, P, D1], bf16, isOutput=False)
    eftl_d = nc.declare_dram_parameter("eftl", [P, T_TILES * ct * P], bf16, isOutput=False)
    dstl_d = nc.declare_dram_parameter("dstl", [P, T_TILES, ct], bf16, isOutput=False)
    # node features blocked [partition, tile, k, node-in-tile]: a group's DMA
    # is one 4KB-per-partition contiguous read (descriptor-efficient)
    nfg_d = nc.declare_dram_parameter("nfg", [P, T_TILES, KD, P], bf16, isOutput=False)
    w1_d = nc.declare_dram_parameter("w1", [P, KD1 * MH * P], bf16, isOutput=False)
    w2_d = nc.declare_dram_parameter("w2", [P, KH * MH * P], bf16, isOutput=False)
    w3_d = nc.declare_dram_parameter("w3", [P, KH * OUT], bf16, isOutput=False)
    # cstB: b1T(MH) | b2T(MH); cstLN: gamma(OUT) | beta(OUT) | b3(OUT) | eps(1)
    cstB_d = nc.declare_dram_parameter("cstB", [P, 2 * MH], f32, isOutput=False)
    cstLN_d = nc.declare_dram_parameter("cstLN", [P, 3 * OUT + 1], f32, isOutput=False)
    cstb_d = nc.declare_dram_parameter("cstb", [P, P], bf16, isOutput=False)
    y_d = nc.declare_dram_parameter("y", [NPAD, OUT], f32, isOutput=True)

    # tile 48 (the ragged one) runs first, then group sizes ramp 2, 3, 4...:
    # the start of the kernel is HBM-bound, so early groups keep their input
    # working set small enough to arrive in time.
    groups = [(T_TILES - 1, 1)]
    t0 = 0
    ramp = [2, 3]
    while t0 < T_TILES - 1:
        g = min(ramp.pop(0) if ramp else GMAX, T_TILES - 1 - t0)
        groups.append((t0, g))
        t0 += g

    with tile.TileContext(nc) as tc:
        with (
            tc.tile_pool(name="const", bufs=1) as constp,
            tc.tile_pool(name="lay", bufs=6) as layp,
            tc.tile_pool(name="lay2", bufs=4) as lay2p,
            tc.tile_pool(name="lay3", bufs=4) as lay3p,
            tc.tile_pool(name="ef", bufs=6) as efp,
            tc.tile_pool(name="oh", bufs=6) as ohp,
            tc.tile_pool(name="ag32", bufs=2) as ag32p,
            tc.tile_pool(name="agg", bufs=3) as aggp,
            tc.tile_pool(name="nfx", bufs=3) as nfxp,
            tc.tile_pool(name="h1", bufs=2) as h1p,
            tc.tile_pool(name="h2", bufs=2) as h2p,
            tc.tile_pool(name="yo", bufs=3) as yop,
            tc.tile_pool(name="st", bufs=8) as stp,
            tc.tile_pool(name="psA", bufs=2, space="PSUM") as psA,
            tc.tile_pool(name="psM", bufs=3, space="PSUM") as psM,
            tc.tile_pool(name="psY", bufs=3, space="PSUM") as psY,
        ):
            # small constants first so the tail path of group 0 clears quickly
            dstl_sb = constp.tile([P, T_TILES, ct], bf16)
            nc.sync.dma_start(out=dstl_sb[:], in_=dstl_d[:, :, :])
            cstb_sb = constp.tile([P, P], bf16)
            nc.sync.dma_start(out=cstb_sb[:], in_=cstb_d[:, :])
            cstB_sb = constp.tile([P, 2 * MH], f32)
            nc.sync.dma_start(out=cstB_sb[:], in_=cstB_d[:, :])
            # group 0's slice of the tail-edge table as its own tiny tile so
            # its one-hot matmuls never wait on the full-table load
            t00 = groups[0][0]
            eftl0_sb = constp.tile([P, ct * P], bf16)
            nc.sync.dma_start(out=eftl0_sb[:],
                              in_=eftl_d[:, t00 * ct * P:(t00 + 1) * ct * P])
            w1_sb = constp.tile([P, MH * KD1 * P], bf16)
            w2_sb = constp.tile([P, MH * KH * P], bf16)
            w3_sb = constp.tile([P, KH * OUT], bf16)
            cstLN_sb = constp.tile([P, 3 * OUT + 1], f32)
            # whole tail-edge table stays resident (12.5KB/partition); group
            # 1's slice loads at startup, the remainder during group 1's body
            eftl_sb = constp.tile([P, T_TILES * ct * P], bf16)
            t10, g1n = groups[1]
            nc.sync.dma_start(
                out=eftl_sb[:, t10 * ct * P:(t10 + g1n) * ct * P],
                in_=eftl_d[:, t10 * ct * P:(t10 + g1n) * ct * P])

            def emit_agg_dmas(tstart, g):
                """Issue the edge-data DMAs for a group one group ahead of
                their compute so the reduces never wait on them."""
                lays = []
                for s in range(g):
                    lay = layp.tile([EDGE_DIM, P, D1], bf16, tag="lay", name="lay")
                    nc.sync.dma_start(out=lay[:], in_=eflay_d[tstart + s, :, :, :])
                    lays.append(lay)
                return lays

            def alloc_agg():
                aggT = aggp.tile([EDGE_DIM, GMAX * P], bf16, tag="agg",
                                 name="aggT")
                agg32 = ag32p.tile([EDGE_DIM, GMAX * P], f32, tag="ag32",
                                   name="agg32")
                ps_t = psA.tile([P, GMAX * P], f32, tag="psA")
                return aggT, agg32, ps_t

            def emit_onehot(t):
                """One-hot of a tile's tail-edge destinations; emitted well
                ahead of the tail matmul so the PE FIFO never waits on it."""
                oh_t = ohp.tile([P, ct, P], bf16, tag="oh", name="oh_t")
                nc.vector.tensor_tensor(
                    out=oh_t[:],
                    in0=dstl_sb[:, t, :, None].to_broadcast([P, ct, P]),
                    in1=cstb_sb[:, None, 0:P].to_broadcast([P, ct, P]),
                    op=Alu.is_equal,
                )
                return oh_t

            def emit_agg_chain(t, s, lay, oh_t, aggT, agg32, ps_t, ef_src=None):
                """Segment-sum of node tile t into aggT[:, s*P:(s+1)*P].

                Vector engine does the bulk (degree-layered reduce); TensorE
                adds the high-degree tail via ct one-hot chunk matmuls into a
                PSUM bank shared by the whole group (no cross-subtile PSUM
                dependency)."""
                for j in range(ct):
                    if ef_src is None:
                        lhsT = eftl_sb[:, (t * ct + j) * P:(t * ct + j + 1) * P]
                    else:
                        lhsT = ef_src[:, j * P:(j + 1) * P]
                    nc.tensor.matmul(
                        out=ps_t[:, s * P:(s + 1) * P],
                        lhsT=lhsT,
                        rhs=oh_t[:, j, :],
                        start=(j == 0),
                        stop=(j == ct - 1),
                    )
                lay2 = lay2p.tile([EDGE_DIM, P, D2], bf16, tag="lay2")
                nc.vector.tensor_tensor(
                    out=lay2[:], in0=lay[:, :, 0:D2], in1=lay[:, :, D2:D1],
                    op=Alu.add,
                )
                lay3 = lay3p.tile([EDGE_DIM, P, D4], bf16, tag="lay3")
                nc.vector.tensor_tensor(
                    out=lay3[:], in0=lay2[:, :, 0:D4], in1=lay2[:, :, D4:D2],
                    op=Alu.add,
                )
                nc.vector.tensor_reduce(
                    out=agg32[:, s * P:(s + 1) * P],
                    in_=lay3[:], axis=Ax.X, op=Alu.add,
                )
                # combine tail + cast to bf16 for the L1 matmul rhs
                nc.vector.tensor_tensor(
                    out=aggT[:, s * P:(s + 1) * P],
                    in0=agg32[:, s * P:(s + 1) * P],
                    in1=ps_t[0:EDGE_DIM, s * P:(s + 1) * P],
                    op=Alu.add,
                )

            def emit_nfx_dma(tstart, g):
                nfx = nfxp.tile([P, GMAX, KD, P], bf16, tag="nfx")
                nc.sync.dma_start(out=nfx[:, 0:g, :, :],
                                  in_=nfg_d[:, tstart:tstart + g, :, :])
                return nfx

            # group 0's aggregation up front (weight DMAs stream in behind it)
            agg_tiles = {}
            agg_dmas = {}
            nfx_tiles = {}
            agg_dmas[0] = emit_agg_dmas(*groups[0])
            nfx_tiles[0] = emit_nfx_dma(*groups[0])
            t00_, g00 = groups[0]
            nc.sync.dma_start(out=w1_sb[:, 0:KD1 * P], in_=w1_d[:, 0:KD1 * P])
            # L1 m=0's node-feature accumulation goes first in the PE queue:
            # it only needs nfx + the w1 m=0 slice, so TensorE starts several
            # us before the tail matmul's inputs land
            g0_ps_m0 = psM.tile([P, GMAX * P], f32, tag="psM")
            for k in range(KD):
                nc.tensor.matmul(
                    out=g0_ps_m0[:, 0:g00 * P],
                    lhsT=w1_sb[:, k * P:(k + 1) * P],
                    rhs=nfx_tiles[0][:, 0:g00, k, :],
                    start=(k == 0),
                    stop=False,
                )
            agg_tiles[0] = alloc_agg()
            for s in range(g00):
                emit_agg_chain(t00_ + s, s, agg_dmas[0][s],
                               emit_onehot(t00_ + s), *agg_tiles[0],
                               ef_src=eftl0_sb)
            agg_dmas.pop(0)
            for m in range(1, MH):
                nc.sync.dma_start(
                    out=w1_sb[:, m * KD1 * P:(m + 1) * KD1 * P],
                    in_=w1_d[:, m * KD1 * P:(m + 1) * KD1 * P])

            for gi, (tstart, g) in enumerate(groups):
                nt = g * P  # free-dim width of this super-tile
                n0 = tstart * P
                aggT, _, _ = agg_tiles.pop(gi)
                nfx = nfx_tiles.pop(gi)

                # next group's input DMAs go out a full group early (except
                # behind group 0's weight DMAs: the ramp is HBM-bound and L1
                # needs w1 slices before any of group 1's edge data)
                if gi == 1:
                    # rest of the tail-edge table (tiles outside groups 0/1)
                    if t10 > 0:
                        nc.sync.dma_start(out=eftl_sb[:, 0:t10 * ct * P],
                                          in_=eftl_d[:, 0:t10 * ct * P])
                    if (t10 + g1n) < t00:
                        nc.sync.dma_start(
                            out=eftl_sb[:, (t10 + g1n) * ct * P:t00 * ct * P],
                            in_=eftl_d[:, (t10 + g1n) * ct * P:t00 * ct * P])
                if gi + 1 < len(groups) and gi > 0:
                    agg_dmas[gi + 1] = emit_agg_dmas(*groups[gi + 1])
                    nfx_tiles[gi + 1] = emit_nfx_dma(*groups[gi + 1])
                if gi == 0:
                    # per-m weight slices stream in behind group 0's agg work so
                    # layer 1/2 can begin as soon as their own slice lands
                    for m in range(MH):
                        nc.sync.dma_start(
                            out=w1_sb[:, m * KD1 * P:(m + 1) * KD1 * P],
                            in_=w1_d[:, m * KD1 * P:(m + 1) * KD1 * P])

                # ---- layer 1: h1T[m] = relu(W1.T @ xT + b1), x = [nf; agg] ----
                h1 = h1p.tile([P, KH, GMAX * P], bf16, tag="h1")
                for m in range(MH):
                    ps = psM.tile([P, GMAX * P], f32, tag="psM")
                    for k in range(KD):
                        nc.tensor.matmul(
                            out=ps[:, 0:nt],
                            lhsT=w1_sb[:, (m * KD1 + k) * P:(m * KD1 + k + 1) * P],
                            rhs=nfx[:, 0:g, k, :],
                            start=(k == 0),
                            stop=False,
                        )
                    nc.tensor.matmul(
                        out=ps[:, 0:nt],
                        lhsT=w1_sb[0:EDGE_DIM, (m * KD1 + KD) * P:(m * KD1 + KD) * P + P],
                        rhs=aggT[:, 0:nt],
                        start=False,
                        stop=True,
                    )
                    nc.scalar.activation(
                        out=h1[:, m, 0:nt], in_=ps[:, 0:nt], func=Act.Relu,
                        bias=cstB_sb[:, m:m + 1],
                    )
                    if gi == 0:
                        nc.sync.dma_start(
                            out=w2_sb[:, m * KH * P:(m + 1) * KH * P],
                            in_=w2_d[:, m * KH * P:(m + 1) * KH * P])
                        # interleave group 1's edge/node data between the w2
                        # slices: its reduces start during this group's L2,
                        # well before w2's tail is needed
                        if m == 1 and len(groups) > 1:
                            agg_dmas[1] = emit_agg_dmas(*groups[1])
                        if m == 3 and len(groups) > 1:
                            nfx_tiles[1] = emit_nfx_dma(*groups[1])

                # ---- layer 2 ----
                h2 = h2p.tile([P, KH, GMAX * P], bf16, tag="h2")
                for m in range(MH):
                    ps = psM.tile([P, GMAX * P], f32, tag="psM")
                    for k in range(KH):
                        nc.tensor.matmul(
                            out=ps[:, 0:nt],
                            lhsT=w2_sb[:, (m * KH + k) * P:(m * KH + k + 1) * P],
                            rhs=h1[:, k, 0:nt],
                            start=(k == 0),
                            stop=(k == KH - 1),
                        )
                    nc.scalar.activation(
                        out=h2[:, m, 0:nt], in_=ps[:, 0:nt], func=Act.Relu,
                        bias=cstB_sb[:, MH + m:MH + m + 1],
                    )
                    if gi == 0 and m < 2:
                        if m == 0:
                            nc.sync.dma_start(out=w3_sb[:], in_=w3_d[:, :])
                        else:
                            nc.sync.dma_start(out=cstLN_sb[:], in_=cstLN_d[:, :])

                # ---- layer 3 (nodes on partitions) + LayerNorm ----
                # the next group's aggregation interleaves per subtile, each
                # chain emitted AFTER that subtile's LN ops so the Vector FIFO
                # never delays the LN chain (psY slack absorbs the rest)
                if gi + 1 < len(groups):
                    tstart_nx, g_nx = groups[gi + 1]
                    agg_tiles[gi + 1] = alloc_agg()
                    lays_nx = agg_dmas.pop(gi + 1)
                    ohs_nx = [emit_onehot(tstart_nx + s) for s in range(g_nx)]
                else:
                    tstart_nx, g_nx = 0, 0
                for s in range(max(g, g_nx)):
                    if s >= g:
                        emit_agg_chain(tstart_nx + s, s, lays_nx[s],
                                       ohs_nx[s], *agg_tiles[gi + 1])
                        continue
                    ps_y = psY.tile([P, OUT], f32, tag="psY")
                    for k in range(KH):
                        nc.tensor.matmul(
                            out=ps_y[:],
                            lhsT=h2[:, k, s * P:(s + 1) * P],
                            rhs=w3_sb[:, k * OUT:(k + 1) * OUT],
                            start=(k == 0),
                            stop=(k == KH - 1),
                        )
                    # + b3 (broadcast rows) on VectorE, off the TensorE critical path
                    nc.vector.tensor_tensor(
                        out=ps_y[:], in0=ps_y[:],
                        in1=cstLN_sb[:, 2 * OUT:3 * OUT],
                        op=Alu.add,
                    )
                    st6 = stp.tile([P, 6], f32, tag="st6")
                    nc.vector.bn_stats(st6[:], ps_y[:])
                    mv = stp.tile([P, 2], f32, tag="mv")
                    nc.vector.bn_aggr(mv[:], st6[:])
                    std = stp.tile([P, 1], f32, tag="std")
                    nc.scalar.activation(std[:], mv[:, 1:2], Act.Sqrt,
                                         bias=cstLN_sb[:, 3 * OUT:])
                    rstd = stp.tile([P, 1], f32, tag="rstd")
                    nc.vector.reciprocal(rstd[:], std[:])
                    nmr = stp.tile([P, 1], f32, tag="nmr")
                    nc.vector.tensor_scalar(
                        out=nmr[:], in0=mv[:, 0:1], scalar1=rstd[:], scalar2=-1.0,
                        op0=Alu.mult, op1=Alu.mult,
                    )
                    yn = yop.tile([P, OUT], f32, tag="yn")
                    nc.scalar.activation(
                        out=yn[:], in_=ps_y[:], func=Act.Identity,
                        bias=nmr[:], scale=rstd[:],
                    )
                    if apply_gamma_beta:
                        nc.vector.tensor_tensor(
                            out=yn[:], in0=yn[:],
                            in1=cstLN_sb[:, 0:OUT], op=Alu.mult,
                        )
                        nc.vector.tensor_tensor(
                            out=yn[:], in0=yn[:],
                            in1=cstLN_sb[:, OUT:2 * OUT], op=Alu.add,
                        )
                    r0 = (tstart + s) * P
                    nc.sync.dma_start(out=y_d[r0:r0 + P, :], in_=yn[:])
                    if s < g_nx:
                        emit_agg_chain(tstart_nx + s, s, lays_nx[s],
                                       ohs_nx[s], *agg_tiles[gi + 1])

    nc.compile()
    return nc


# ----------------------------------------------------------------------------
# Host-side sharding / layout prep
# ----------------------------------------------------------------------------

def _prep_core(c, node_feat, edge_feat, dst, ct):
    KD_ = NODE_DIM // P
    lo = c * NPC
    sel = np.flatnonzero((dst >= lo) & (dst < lo + NPC))
    d = (dst[sel] - lo).astype(np.int64)
    order = np.argsort(d, kind="stable")
    sel = sel[order]
    d = d[order]
    counts = np.bincount(d, minlength=NPAD)
    offs = np.zeros(NPAD, np.int64)
    np.cumsum(counts[:-1], out=offs[1:])
    rank = np.arange(d.size) - offs[d]

    # main: first D1 edges of each node, degree-layered [T, 96, 128, D1]
    main = rank < D1
    flat = np.zeros((NPAD * D1, EDGE_DIM), np.float32)
    flat[d[main] * D1 + rank[main]] = edge_feat[sel[main]]
    eflay = np.ascontiguousarray(
        flat.astype(BF16).reshape(T_TILES, P, D1, EDGE_DIM).transpose(0, 3, 1, 2))

    # tail: edges beyond D1 per node, chunked one-hot layout per tile
    tail = np.flatnonzero(rank >= D1)
    dt_ = d[tail]
    tile_of = dt_ >> 7
    tcounts = np.bincount(tile_of, minlength=T_TILES)
    toffs = np.zeros(T_TILES, np.int64)
    np.cumsum(tcounts[:-1], out=toffs[1:])
    trank = np.arange(dt_.size) - toffs[tile_of]
    p_slot = trank % P
    c_slot = trank // P
    assert c_slot.max(initial=0) < ct

    eftl = np.zeros((T_TILES, P, ct, P), BF16)
    eftl[tile_of, p_slot, c_slot, :EDGE_DIM] = edge_feat[sel[tail]].astype(BF16)
    # resident layout: [partition(edge slot), tile*chunk*feat]
    eftl = np.ascontiguousarray(eftl.transpose(1, 0, 2, 3)).reshape(P, -1)
    dstl = np.full((T_TILES, P, ct), -1.0, BF16)
    dstl[tile_of, p_slot, c_slot] = (dt_ - (tile_of << 7)).astype(BF16)
    dstl = np.ascontiguousarray(dstl.transpose(1, 0, 2))

    # node features blocked [partition, tile, k, node-in-tile]:
    # nfg[p, t, k, j] = node_feat[t*128+j, k*128+p]
    nfp = np.zeros((NPAD, NODE_DIM), np.float32)
    nfp[:NPC] = node_feat[lo:lo + NPC]
    nfg = np.ascontiguousarray(
        nfp.astype(BF16).reshape(T_TILES, P, KD_, P).transpose(3, 0, 2, 1))
    return {"eflay": eflay, "eftl": eftl, "dstl": dstl, "nfg": nfg}


def _prep_shared(W1, b1, W2, b2, W3, b3, gamma, beta):
    KD1 = NODE_DIM // P + 1
    MH = HID // P
    KH = HID // P

    w1p = np.zeros((KD1 * P, HID), np.float32)
    w1p[:NODE_DIM + EDGE_DIM] = W1
    # m-major: col index (m*KD1 + k)*P + j
    w1 = np.ascontiguousarray(
        w1p.reshape(KD1, P, MH, P).transpose(1, 2, 0, 3)).reshape(P, -1).astype(BF16)
    w2 = np.ascontiguousarray(
        W2.reshape(KH, P, MH, P).transpose(1, 2, 0, 3)).reshape(P, -1).astype(BF16)
    w3 = np.ascontiguousarray(
        W3.reshape(KH, P, OUT).transpose(1, 0, 2)).reshape(P, -1).astype(BF16)

    cstB = np.ascontiguousarray(np.concatenate(
        [b1.reshape(MH, P).T, b2.reshape(MH, P).T], axis=1).astype(np.float32))
    cstLN = np.ascontiguousarray(np.concatenate([
        np.tile(gamma.reshape(1, OUT), (P, 1)),
        np.tile(beta.reshape(1, OUT), (P, 1)),
        np.tile(b3.reshape(1, OUT), (P, 1)),
        np.full((P, 1), LN_EPS, np.float32),
    ], axis=1).astype(np.float32))

    cstb = np.tile(np.arange(P, dtype=np.float32)[None, :], (P, 1)).astype(BF16)
    return {"w1": w1, "w2": w2, "w3": w3, "cstB": cstB, "cstLN": cstLN, "cstb": cstb}


# ----------------------------------------------------------------------------
# Entry point
# ----------------------------------------------------------------------------

def _ensure_axon_hooks_importable():
    """bass_utils imports antenv.axon_hooks when tracing is requested (even via
    the BASS_TRACE env var); provide a no-op stub if the module is absent so
    that path degrades to trace-skipped instead of crashing."""
    try:
        import antenv.axon_hooks  # noqa: F401
    except Exception:
        import sys
        import types
        try:
            import antenv
        except Exception:
            return
        mod = types.ModuleType('antenv.axon_hooks')
        mod._hook = None
        mod.set_axon_ntff_profile_hook = lambda h: setattr(mod, '_hook', h)
        mod.get_axon_ntff_profile_hook = lambda: mod._hook
        sys.modules['antenv.axon_hooks'] = mod
        antenv.axon_hooks = mod


def kernel(node_feat, edge_feat, edge_index, n_nodes, W1, b1, W2, b2, W3, b3,
           gamma, beta, _want_trace=False):
    from concourse.bass_utils import run_bass_kernel_spmd
    _ensure_axon_hooks_importable()

    node_feat = np.asarray(node_feat, dtype=np.float32)
    edge_feat = np.asarray(edge_feat, dtype=np.float32)
    edge_index = np.asarray(edge_index)
    assert int(n_nodes) == N_NODES
    assert node_feat.shape == (N_NODES, NODE_DIM)
    assert edge_feat.shape == (N_EDGES, EDGE_DIM)

    dst = edge_index[1].astype(np.int64)

    # tail capacity: chunks of 128 edges per tile beyond D1 per node (global,
    # so the SPMD program is shared by all cores)
    counts = np.bincount(dst, minlength=N_NODES)
    padded = np.zeros((NCORES, NPAD), np.int64)
    padded[:, :NPC] = counts.reshape(NCORES, NPC)
    tail_tile = np.maximum(padded - D1, 0).reshape(NCORES, T_TILES, P).sum(axis=2)
    ct = max(1, -(-int(tail_tile.max()) // P))

    gamma = np.asarray(gamma, dtype=np.float32)
    beta = np.asarray(beta, dtype=np.float32)
    apply_gb = not (np.all(gamma == 1.0) and np.all(beta == 0.0))

    key = (ct, apply_gb)
    if key not in _CACHE:
        _CACHE[key] = _build_program(ct, apply_gb)
    nc = _CACHE[key]

    shared = _prep_shared(
        np.asarray(W1, np.float32), np.asarray(b1, np.float32),
        np.asarray(W2, np.float32), np.asarray(b2, np.float32),
        np.asarray(W3, np.float32), np.asarray(b3, np.float32),
        gamma, beta)

    in_maps = []
    for c in range(NCORES):
        m = _prep_core(c, node_feat, edge_feat, dst, ct)
        m.update(shared)
        in_maps.append(m)

    res = run_bass_kernel_spmd(nc, in_maps, list(range(NCORES)), trace=_want_trace)

    y = np.concatenate([res.results[c]["y"][:NPC] for c in range(NCORES)], axis=0)
    out = np.ascontiguousarray(y, dtype=np.float32)
    if _want_trace:
        kernel.last_results = res
    return out


kernel.last_results = None


# revision 44
# speedup vs baseline: 1.1767x; 1.0046x over previous
"""GNN NodeBlock (message passing + 3-layer MLP + LayerNorm) on 8 Trainium2 cores.

Strategy (data parallel over nodes):
  - Shard 50000 nodes across 8 cores (6250 each, padded to 6272 = 49*128).
  - Segment-sum of edge features runs on the *Vector engine*, not TensorE:
    the host lays edges out degree-layered as eflay[tile, 96f, 128n, D1]
    (layer d = d-th incoming edge of each node, zero-padded), which the DVE
    reduces with two bf16 halving adds (2x mode) + one fp32 tensor_reduce.
    Nodes with degree > D1 spill into a tiny per-tile one-hot matmul tail
    (capacity CT chunks of 128 edges) on TensorE, combined during the cast
    to bf16. This frees ~55us of TensorE time vs an all-matmul segment sum.
  - The MLP runs entirely in T-layout (features on partitions, nodes on the
    free dim) with weights stationary: h^T = W.T @ x^T, so no transposes are
    needed between layers. Node features enter pre-transposed from the host.
  - Layer 3 swaps the operands (activations stationary) to produce y in natural
    layout [128 nodes, 512 feats]; bias b3 is added on VectorE. LayerNorm
    reduces over the free dim: bn_stats/bn_aggr (VectorE) + sqrt (ScalarE) +
    reciprocal (VectorE), applied via one ScalarE activation with
    per-partition scale/bias.
  - The lone 49th tile runs as the FIRST group (small working set => early
    TensorE start) so the drain ends on a dense 4-tile group.
  - All matmuls are bf16 inputs with fp32 PSUM accumulation (~4e-3 L2 rel err).

Everything is compiled once per (CT, apply_gamma_beta) configuration and cached.
"""

import numpy as np
import ml_dtypes

P = 128
NODE_DIM = 512
EDGE_DIM = 96
HID = 1024
OUT = 512
N_NODES = 50000
N_EDGES = 800000
NCORES = 8
LN_EPS = 1e-5

NPC = N_NODES // NCORES          # 6250 nodes per core
T_TILES = -(-NPC // P)           # 49 node tiles per core
NPAD = T_TILES * P               # 6272
GMAX = 4                         # node tiles per super-tile (NT = 512 free dim)
D1 = 20                          # degree layers summed on the Vector engine

BF16 = ml_dtypes.bfloat16

_CACHE: dict = {}


# ----------------------------------------------------------------------------
# Bass program
# ----------------------------------------------------------------------------

def _build_program(ct: int, apply_gamma_beta: bool):
    import concourse.bass as bass
    import concourse.bacc as bacc
    import concourse.mybir as mybir
    import concourse.tile as tile

    f32 = mybir.dt.float32
    bf16 = mybir.dt.bfloat16
    Act = mybir.ActivationFunctionType
    Alu = mybir.AluOpType
    Ax = mybir.AxisListType

    KD = NODE_DIM // P           # 4 node-feat k-chunks
    KH = HID // P                # 8 hidden k-chunks
    MH = HID // P                # 8 hidden m-chunks
    KD1 = KD + 1                 # + 1 chunk for the 96 agg features
    D2 = D1 // 2                 # 10
    D4 = D1 // 4                 # 5

    nc = bacc.Bacc("TRN2", target_bir_lowering=False, debug=False)

    # inputs (per core)
    eflay_d = nc.declare_dram_parameter("eflay", [T_TILES, EDGE_DIM, P, D1], bf16, isOutput=False)
    eftl_d = nc.declare_dram_parameter("eftl", [P, T_TILES * ct * P], bf16, isOutput=False)
    dstl_d = nc.declare_dram_parameter("dstl", [P, T_TILES, ct], bf16, isOutput=False)
    # node features blocked [partition, tile, k, node-in-tile]: a group's DMA
    # is one 4KB-per-partition contiguous read (descriptor-efficient)
    nfg_d = nc.declare_dram_parameter("nfg", [P, T_TILES, KD, P], bf16, isOutput=False)
    w1_d = nc.declare_dram_parameter("w1", [P, KD1 * MH * P], bf16, isOutput=False)
    w2_d = nc.declare_dram_parameter("w2", [P, KH * MH * P], bf16, isOutput=False)
    w3_d = nc.declare_dram_parameter("w3", [P, KH * OUT], bf16, isOutput=False)
    # cstB: b1T(MH) | b2T(MH); cstLN: gamma(OUT) | beta(OUT) | b3(OUT) | eps(1)
    cstB_d = nc.declare_dram_parameter("cstB", [P, 2 * MH], f32, isOutput=False)
    cstLN_d = nc.declare_dram_parameter("cstLN", [P, 3 * OUT + 1], f32, isOutput=False)
    cstb_d = nc.declare_dram_parameter("cstb", [P, P], bf16, isOutput=False)
    y_d = nc.declare_dram_parameter("y", [NPAD, OUT], f32, isOutput=True)

    # tile 48 (the ragged one) runs first, then group sizes ramp 2, 3, 4...:
    # the start of the kernel is HBM-bound, so early groups keep their input
    # working set small enough to arrive in time.
    groups = [(T_TILES - 1, 1)]
    t0 = 0
    ramp = [2, 3]
    while t0 < T_TILES - 1:
        g = min(ramp.pop(0) if ramp else GMAX, T_TILES - 1 - t0)
        groups.append((t0, g))
        t0 += g

    with tile.TileContext(nc) as tc:
        with (
            tc.tile_pool(name="const", bufs=1) as constp,
            tc.tile_pool(name="lay", bufs=6) as layp,
            tc.tile_pool(name="lay2", bufs=4) as lay2p,
            tc.tile_pool(name="lay3", bufs=4) as lay3p,
            tc.tile_pool(name="ef", bufs=6) as efp,
            tc.tile_pool(name="oh", bufs=6) as ohp,
            tc.tile_pool(name="ag32", bufs=2) as ag32p,
            tc.tile_pool(name="agg", bufs=3) as aggp,
            tc.tile_pool(name="nfx", bufs=3) as nfxp,
            tc.tile_pool(name="h1", bufs=2) as h1p,
            tc.tile_pool(name="h2", bufs=2) as h2p,
            tc.tile_pool(name="yo", bufs=3) as yop,
            tc.tile_pool(name="st", bufs=8) as stp,
            tc.tile_pool(name="psA", bufs=2, space="PSUM") as psA,
            tc.tile_pool(name="psM", bufs=3, space="PSUM") as psM,
            tc.tile_pool(name="psY", bufs=3, space="PSUM") as psY,
        ):
            # small constants first so the tail path of group 0 clears quickly
            dstl_sb = constp.tile([P, T_TILES, ct], bf16)
            nc.sync.dma_start(out=dstl_sb[:], in_=dstl_d[:, :, :])
            cstb_sb = constp.tile([P, P], bf16)
            nc.sync.dma_start(out=cstb_sb[:], in_=cstb_d[:, :])
            cstB_sb = constp.tile([P, 2 * MH], f32)
            nc.sync.dma_start(out=cstB_sb[:], in_=cstB_d[:, :])
            # group 0's slice of the tail-edge table as its own tiny tile so
            # its one-hot matmuls never wait on the full-table load
            t00 = groups[0][0]
            eftl0_sb = constp.tile([P, ct * P], bf16)
            nc.sync.dma_start(out=eftl0_sb[:],
                              in_=eftl_d[:, t00 * ct * P:(t00 + 1) * ct * P])
            w1_sb = constp.tile([P, MH * KD1 * P], bf16)
            w2_sb = constp.tile([P, MH * KH * P], bf16)
            w3_sb = constp.tile([P, KH * OUT], bf16)
            cstLN_sb = constp.tile([P, 3 * OUT + 1], f32)
            # whole tail-edge table stays resident (12.5KB/partition); group
            # 1's slice loads at startup, the remainder during group 1's body
            eftl_sb = constp.tile([P, T_TILES * ct * P], bf16)
            t10, g1n = groups[1]
            nc.sync.dma_start(
                out=eftl_sb[:, t10 * ct * P:(t10 + g1n) * ct * P],
                in_=eftl_d[:, t10 * ct * P:(t10 + g1n) * ct * P])

            def emit_agg_dmas(tstart, g):
                """Issue the edge-data DMAs for a group one group ahead of
                their compute so the reduces never wait on them."""
                lays = []
                for s in range(g):
                    lay = layp.tile([EDGE_DIM, P, D1], bf16, tag="lay", name="lay")
                    nc.sync.dma_start(out=lay[:], in_=eflay_d[tstart + s, :, :, :])
                    lays.append(lay)
                return lays

            def alloc_agg():
                aggT = aggp.tile([EDGE_DIM, GMAX * P], bf16, tag="agg",
                                 name="aggT")
                agg32 = ag32p.tile([EDGE_DIM, GMAX * P], f32, tag="ag32",
                                   name="agg32")
                ps_t = psA.tile([P, GMAX * P], f32, tag="psA")
                return aggT, agg32, ps_t

            def emit_onehot(t):
                """One-hot of a tile's tail-edge destinations; emitted well
                ahead of the tail matmul so the PE FIFO never waits on it."""
                oh_t = ohp.tile([P, ct, P], bf16, tag="oh", name="oh_t")
                nc.vector.tensor_tensor(
                    out=oh_t[:],
                    in0=dstl_sb[:, t, :, None].to_broadcast([P, ct, P]),
                    in1=cstb_sb[:, None, 0:P].to_broadcast([P, ct, P]),
                    op=Alu.is_equal,
                )
                return oh_t

            def emit_agg_chain(t, s, lay, oh_t, aggT, agg32, ps_t, ef_src=None):
                """Segment-sum of node tile t into aggT[:, s*P:(s+1)*P].

                Vector engine does the bulk (degree-layered reduce); TensorE
                adds the high-degree tail via ct one-hot chunk matmuls into a
                PSUM bank shared by the whole group (no cross-subtile PSUM
                dependency)."""
                for j in range(ct):
                    if ef_src is None:
                        lhsT = eftl_sb[:, (t * ct + j) * P:(t * ct + j + 1) * P]
                    else:
                        lhsT = ef_src[:, j * P:(j + 1) * P]
                    nc.tensor.matmul(
                        out=ps_t[:, s * P:(s + 1) * P],
                        lhsT=lhsT,
                        rhs=oh_t[:, j, :],
                        start=(j == 0),
                        stop=(j == ct - 1),
                    )
                lay2 = lay2p.tile([EDGE_DIM, P, D2], bf16, tag="lay2")
                nc.vector.tensor_tensor(
                    out=lay2[:], in0=lay[:, :, 0:D2], in1=lay[:, :, D2:D1],
                    op=Alu.add,
                )
                lay3 = lay3p.tile([EDGE_DIM, P, D4], bf16, tag="lay3")
                nc.vector.tensor_tensor(
                    out=lay3[:], in0=lay2[:, :, 0:D4], in1=lay2[:, :, D4:D2],
                    op=Alu.add,
                )
                nc.vector.tensor_reduce(
                    out=agg32[:, s * P:(s + 1) * P],
                    in_=lay3[:], axis=Ax.X, op=Alu.add,
                )
                # combine tail + cast to bf16 for the L1 matmul rhs
                nc.vector.tensor_tensor(
                    out=aggT[:, s * P:(s + 1) * P],
                    in0=agg32[:, s * P:(s + 1) * P],
                    in1=ps_t[0:EDGE_DIM, s * P:(s + 1) * P],
                    op=Alu.add,
                )

            def emit_nfx_dma(tstart, g):
                nfx = nfxp.tile([P, GMAX, KD, P], bf16, tag="nfx")
                nc.sync.dma_start(out=nfx[:, 0:g, :, :],
                                  in_=nfg_d[:, tstart:tstart + g, :, :])
                return nfx

            # group 0's aggregation up front (weight DMAs stream in behind it)
            agg_tiles = {}
            agg_dmas = {}
            nfx_tiles = {}
            agg_dmas[0] = emit_agg_dmas(*groups[0])
            nfx_tiles[0] = emit_nfx_dma(*groups[0])
            t00_, g00 = groups[0]
            nc.sync.dma_start(out=w1_sb[:, 0:KD1 * P], in_=w1_d[:, 0:KD1 * P])
            # L1 m=0's node-feature accumulation goes first in the PE queue:
            # it only needs nfx + the w1 m=0 slice, so TensorE starts several
            # us before the tail matmul's inputs land
            g0_ps_m0 = psM.tile([P, GMAX * P], f32, tag="psM")
            for k in range(KD):
                nc.tensor.matmul(
                    out=g0_ps_m0[:, 0:g00 * P],
                    lhsT=w1_sb[:, k * P:(k + 1) * P],
                    rhs=nfx_tiles[0][:, 0:g00, k, :],
                    start=(k == 0),
                    stop=False,
                )
            agg_tiles[0] = alloc_agg()
            for s in range(g00):
                emit_agg_chain(t00_ + s, s, agg_dmas[0][s],
                               emit_onehot(t00_ + s), *agg_tiles[0],
                               ef_src=eftl0_sb)
            agg_dmas.pop(0)
            for m in range(1, MH):
                nc.sync.dma_start(
                    out=w1_sb[:, m * KD1 * P:(m + 1) * KD1 * P],
                    in_=w1_d[:, m * KD1 * P:(m + 1) * KD1 * P])

            for gi, (tstart, g) in enumerate(groups):
                nt = g * P  # free-dim width of this super-tile
                n0 = tstart * P
                aggT, _, _ = agg_tiles.pop(gi)
                nfx = nfx_tiles.pop(gi)

                # next group's input DMAs go out a full group early (except
                # behind group 0's weight DMAs: the ramp is HBM-bound and L1
                # needs w1 slices before any of group 1's edge data)
                if gi == 1:
                    # rest of the tail-edge table (tiles outside groups 0/1)
                    if t10 > 0:
                        nc.sync.dma_start(out=eftl_sb[:, 0:t10 * ct * P],
                                          in_=eftl_d[:, 0:t10 * ct * P])
                    if (t10 + g1n) < t00:
                        nc.sync.dma_start(
                            out=eftl_sb[:, (t10 + g1n) * ct * P:t00 * ct * P],
                            in_=eftl_d[:, (t10 + g1n) * ct * P:t00 * ct * P])
                if gi + 1 < len(groups) and gi > 0:
                    agg_dmas[gi + 1] = emit_agg_dmas(*groups[gi + 1])
                    nfx_tiles[gi + 1] = emit_nfx_dma(*groups[gi + 1])

                # ---- layer 1: h1T[m] = relu(W1.T @ xT + b1), x = [nf; agg] ----
                h1 = h1p.tile([P, KH, GMAX * P], bf16, tag="h1")
                for m in range(MH):
                    if gi == 0 and m == 0:
                        # node-feature chunks were pre-emitted before group 0's
                        # agg chains; only the agg chunk remains
                        ps = g0_ps_m0
                    else:
                        ps = psM.tile([P, GMAX * P], f32, tag="psM")
                        for k in range(KD):
                            nc.tensor.matmul(
                                out=ps[:, 0:nt],
                                lhsT=w1_sb[:, (m * KD1 + k) * P:(m * KD1 + k + 1) * P],
                                rhs=nfx[:, 0:g, k, :],
                                start=(k == 0),
                                stop=False,
                            )
                    nc.tensor.matmul(
                        out=ps[:, 0:nt],
                        lhsT=w1_sb[0:EDGE_DIM, (m * KD1 + KD) * P:(m * KD1 + KD) * P + P],
                        rhs=aggT[:, 0:nt],
                        start=False,
                        stop=True,
                    )
                    nc.scalar.activation(
                        out=h1[:, m, 0:nt], in_=ps[:, 0:nt], func=Act.Relu,
                        bias=cstB_sb[:, m:m + 1],
                    )
                    if gi == 0:
                        nc.sync.dma_start(
                            out=w2_sb[:, m * KH * P:(m + 1) * KH * P],
                            in_=w2_d[:, m * KH * P:(m + 1) * KH * P])
                        # interleave group 1's edge/node data between the w2
                        # slices: its reduces start during this group's L2,
                        # well before w2's tail is needed
                        if m == 1 and len(groups) > 1:
                            agg_dmas[1] = emit_agg_dmas(*groups[1])
                        if m == 3 and len(groups) > 1:
                            nfx_tiles[1] = emit_nfx_dma(*groups[1])

                # ---- layer 2 ----
                h2 = h2p.tile([P, KH, GMAX * P], bf16, tag="h2")
                for m in range(MH):
                    ps = psM.tile([P, GMAX * P], f32, tag="psM")
                    for k in range(KH):
                        nc.tensor.matmul(
                            out=ps[:, 0:nt],
                            lhsT=w2_sb[:, (m * KH + k) * P:(m * KH + k + 1) * P],
                            rhs=h1[:, k, 0:nt],
                            start=(k == 0),
                            stop=(k == KH - 1),
                        )
                    nc.scalar.activation(
                        out=h2[:, m, 0:nt], in_=ps[:, 0:nt], func=Act.Relu,
                        bias=cstB_sb[:, MH + m:MH + m + 1],
                    )
                    if gi == 0 and m < 2:
                        if m == 0:
                            nc.sync.dma_start(out=w3_sb[:], in_=w3_d[:, :])
                        else:
                            nc.sync.dma_start(out=cstLN_sb[:], in_=cstLN_d[:, :])

                # ---- layer 3 (nodes on partitions) + LayerNorm ----
                # the next group's aggregation interleaves per subtile, each
                # chain emitted AFTER that subtile's LN ops so the Vector FIFO
                # never delays the LN chain (psY slack absorbs the rest)
                if gi + 1 < len(groups):
                    tstart_nx, g_nx = groups[gi + 1]
                    agg_tiles[gi + 1] = alloc_agg()
                    lays_nx = agg_dmas.pop(gi + 1)
                    ohs_nx = [emit_onehot(tstart_nx + s) for s in range(g_nx)]
                else:
                    tstart_nx, g_nx = 0, 0
                for s in range(max(g, g_nx)):
                    if s >= g:
                        emit_agg_chain(tstart_nx + s, s, lays_nx[s],
                                       ohs_nx[s], *agg_tiles[gi + 1])
                        continue
                    ps_y = psY.tile([P, OUT], f32, tag="psY")
                    for k in range(KH):
                        nc.tensor.matmul(
                            out=ps_y[:],
                            lhsT=h2[:, k, s * P:(s + 1) * P],
                            rhs=w3_sb[:, k * OUT:(k + 1) * OUT],
                            start=(k == 0),
                            stop=(k == KH - 1),
                        )
                    # + b3 (broadcast rows) on VectorE, off the TensorE critical path
                    nc.vector.tensor_tensor(
                        out=ps_y[:], in0=ps_y[:],
                        in1=cstLN_sb[:, 2 * OUT:3 * OUT],
                        op=Alu.add,
                    )
                    st6 = stp.tile([P, 6], f32, tag="st6")
                    nc.vector.bn_stats(st6[:], ps_y[:])
                    mv = stp.tile([P, 2], f32, tag="mv")
                    nc.vector.bn_aggr(mv[:], st6[:])
                    std = stp.tile([P, 1], f32, tag="std")
                    nc.scalar.activation(std[:], mv[:, 1:2], Act.Sqrt,
                                         bias=cstLN_sb[:, 3 * OUT:])
                    rstd = stp.tile([P, 1], f32, tag="rstd")
                    nc.vector.reciprocal(rstd[:], std[:])
                    nmr = stp.tile([P, 1], f32, tag="nmr")
                    nc.vector.tensor_scalar(
                        out=nmr[:], in0=mv[:, 0:1], scalar1=rstd[:], scalar2=-1.0,
                        op0=Alu.mult, op1=Alu.mult,
                    )
                    yn = yop.tile([P, OUT], f32, tag="yn")
                    nc.scalar.activation(
                        out=yn[:], in_=ps_y[:], func=Act.Identity,
                        bias=nmr[:], scale=rstd[:],
                    )
                    if apply_gamma_beta:
                        nc.vector.tensor_tensor(
                            out=yn[:], in0=yn[:],
                            in1=cstLN_sb[:, 0:OUT], op=Alu.mult,
                        )
                        nc.vector.tensor_tensor(
                            out=yn[:], in0=yn[:],
                            in1=cstLN_sb[:, OUT:2 * OUT], op=Alu.add,
                        )
                    r0 = (tstart + s) * P
                    nc.sync.dma_start(out=y_d[r0:r0 + P, :], in_=yn[:])
                    if s < g_nx:
                        emit_agg_chain(tstart_nx + s, s, lays_nx[s],
                                       ohs_nx[s], *agg_tiles[gi + 1])

    nc.compile()
    return nc


# ----------------------------------------------------------------------------
# Host-side sharding / layout prep
# ----------------------------------------------------------------------------

def _prep_core(c, node_feat, edge_feat, dst, ct):
    KD_ = NODE_DIM // P
    lo = c * NPC
    sel = np.flatnonzero((dst >= lo) & (dst < lo + NPC))
    d = (dst[sel] - lo).astype(np.int64)
    order = np.argsort(d, kind="stable")
    sel = sel[order]
    d = d[order]
    counts = np.bincount(d, minlength=NPAD)
    offs = np.zeros(NPAD, np.int64)
    np.cumsum(counts[:-1], out=offs[1:])
    rank = np.arange(d.size) - offs[d]

    # main: first D1 edges of each node, degree-layered [T, 96, 128, D1]
    main = rank < D1
    flat = np.zeros((NPAD * D1, EDGE_DIM), np.float32)
    flat[d[main] * D1 + rank[main]] = edge_feat[sel[main]]
    eflay = np.ascontiguousarray(
        flat.astype(BF16).reshape(T_TILES, P, D1, EDGE_DIM).transpose(0, 3, 1, 2))

    # tail: edges beyond D1 per node, chunked one-hot layout per tile
    tail = np.flatnonzero(rank >= D1)
    dt_ = d[tail]
    tile_of = dt_ >> 7
    tcounts = np.bincount(tile_of, minlength=T_TILES)
    toffs = np.zeros(T_TILES, np.int64)
    np.cumsum(tcounts[:-1], out=toffs[1:])
    trank = np.arange(dt_.size) - toffs[tile_of]
    p_slot = trank % P
    c_slot = trank // P
    assert c_slot.max(initial=0) < ct

    eftl = np.zeros((T_TILES, P, ct, P), BF16)
    eftl[tile_of, p_slot, c_slot, :EDGE_DIM] = edge_feat[sel[tail]].astype(BF16)
    # resident layout: [partition(edge slot), tile*chunk*feat]
    eftl = np.ascontiguousarray(eftl.transpose(1, 0, 2, 3)).reshape(P, -1)
    dstl = np.full((T_TILES, P, ct), -1.0, BF16)
    dstl[tile_of, p_slot, c_slot] = (dt_ - (tile_of << 7)).astype(BF16)
    dstl = np.ascontiguousarray(dstl.transpose(1, 0, 2))

    # node features blocked [partition, tile, k, node-in-tile]:
    # nfg[p, t, k, j] = node_feat[t*128+j, k*128+p]
    nfp = np.zeros((NPAD, NODE_DIM), np.float32)
    nfp[:NPC] = node_feat[lo:lo + NPC]
    nfg = np.ascontiguousarray(
        nfp.astype(BF16).reshape(T_TILES, P, KD_, P).transpose(3, 0, 2, 1))
    return {"eflay": eflay, "eftl": eftl, "dstl": dstl, "nfg": nfg}


def _prep_shared(W1, b1, W2, b2, W3, b3, gamma, beta):
    KD1 = NODE_DIM // P + 1
    MH = HID // P
    KH = HID // P

    w1p = np.zeros((KD1 * P, HID), np.float32)
    w1p[:NODE_DIM + EDGE_DIM] = W1
    # m-major: col index (m*KD1 + k)*P + j
    w1 = np.ascontiguousarray(
        w1p.reshape(KD1, P, MH, P).transpose(1, 2, 0, 3)).reshape(P, -1).astype(BF16)
    w2 = np.ascontiguousarray(
        W2.reshape(KH, P, MH, P).transpose(1, 2, 0, 3)).reshape(P, -1).astype(BF16)
    w3 = np.ascontiguousarray(
        W3.reshape(KH, P, OUT).transpose(1, 0, 2)).reshape(P, -1).astype(BF16)

    cstB = np.ascontiguousarray(np.concatenate(
        [b1.reshape(MH, P).T, b2.reshape(MH, P).T], axis=1).astype(np.float32))
    cstLN = np.ascontiguousarray(np.concatenate([
        np.tile(gamma.reshape(1, OUT), (P, 1)),
        np.tile(beta.reshape(1, OUT), (P, 1)),
        np.tile(b3.reshape(1, OUT), (P, 1)),
        np.full((P, 1), LN_EPS, np.float32),
    ], axis=1).astype(np.float32))

    cstb = np.tile(np.arange(P, dtype=np.float32)[None, :], (P, 1)).astype(BF16)
    return {"w1": w1, "w2": w2, "w3": w3, "cstB": cstB, "cstLN": cstLN, "cstb": cstb}


# ----------------------------------------------------------------------------
# Entry point
# ----------------------------------------------------------------------------

def _ensure_axon_hooks_importable():
    """bass_utils imports antenv.axon_hooks when tracing is requested (even via
    the BASS_TRACE env var); provide a no-op stub if the module is absent so
    that path degrades to trace-skipped instead of crashing."""
    try:
        import antenv.axon_hooks  # noqa: F401
    except Exception:
        import sys
        import types
        try:
            import antenv
        except Exception:
            return
        mod = types.ModuleType('antenv.axon_hooks')
        mod._hook = None
        mod.set_axon_ntff_profile_hook = lambda h: setattr(mod, '_hook', h)
        mod.get_axon_ntff_profile_hook = lambda: mod._hook
        sys.modules['antenv.axon_hooks'] = mod
        antenv.axon_hooks = mod


def kernel(node_feat, edge_feat, edge_index, n_nodes, W1, b1, W2, b2, W3, b3,
           gamma, beta, _want_trace=False):
    from concourse.bass_utils import run_bass_kernel_spmd
    _ensure_axon_hooks_importable()

    node_feat = np.asarray(node_feat, dtype=np.float32)
    edge_feat = np.asarray(edge_feat, dtype=np.float32)
    edge_index = np.asarray(edge_index)
    assert int(n_nodes) == N_NODES
    assert node_feat.shape == (N_NODES, NODE_DIM)
    assert edge_feat.shape == (N_EDGES, EDGE_DIM)

    dst = edge_index[1].astype(np.int64)

    # tail capacity: chunks of 128 edges per tile beyond D1 per node (global,
    # so the SPMD program is shared by all cores)
    counts = np.bincount(dst, minlength=N_NODES)
    padded = np.zeros((NCORES, NPAD), np.int64)
    padded[:, :NPC] = counts.reshape(NCORES, NPC)
    tail_tile = np.maximum(padded - D1, 0).reshape(NCORES, T_TILES, P).sum(axis=2)
    ct = max(1, -(-int(tail_tile.max()) // P))

    gamma = np.asarray(gamma, dtype=np.float32)
    beta = np.asarray(beta, dtype=np.float32)
    apply_gb = not (np.all(gamma == 1.0) and np.all(beta == 0.0))

    key = (ct, apply_gb)
    if key not in _CACHE:
        _CACHE[key] = _build_program(ct, apply_gb)
    nc = _CACHE[key]

    shared = _prep_shared(
        np.asarray(W1, np.float32), np.asarray(b1, np.float32),
        np.asarray(W2, np.float32), np.asarray(b2, np.float32),
        np.asarray(W3, np.float32), np.asarray(b3, np.float32),
        gamma, beta)

    in_maps = []
    for c in range(NCORES):
        m = _prep_core(c, node_feat, edge_feat, dst, ct)
        m.update(shared)
        in_maps.append(m)

    res = run_bass_kernel_spmd(nc, in_maps, list(range(NCORES)), trace=_want_trace)

    y = np.concatenate([res.results[c]["y"][:NPC] for c in range(NCORES)], axis=0)
    out = np.ascontiguousarray(y, dtype=np.float32)
    if _want_trace:
        kernel.last_results = res
    return out


kernel.last_results = None


# revision 48
# speedup vs baseline: 1.1845x; 1.0066x over previous
"""GNN NodeBlock (message passing + 3-layer MLP + LayerNorm) on 8 Trainium2 cores.

Strategy (data parallel over nodes):
  - Shard 50000 nodes across 8 cores (6250 each, padded to 6272 = 49*128).
  - Segment-sum of edge features runs on the *Vector engine*, not TensorE:
    the host lays edges out degree-layered as eflay[tile, 96f, 128n, D1]
    (layer d = d-th incoming edge of each node, zero-padded), which the DVE
    reduces with two bf16 halving adds (2x mode) + one fp32 tensor_reduce.
    Nodes with degree > D1 spill into a tiny per-tile one-hot matmul tail
    (capacity CT chunks of 128 edges) on TensorE, combined during the cast
    to bf16. This frees ~55us of TensorE time vs an all-matmul segment sum.
  - The MLP runs entirely in T-layout (features on partitions, nodes on the
    free dim) with weights stationary: h^T = W.T @ x^T, so no transposes are
    needed between layers. Node features enter pre-transposed from the host.
  - Layer 3 swaps the operands (activations stationary) to produce y in natural
    layout [128 nodes, 512 feats]; bias b3 is added on VectorE. LayerNorm
    reduces over the free dim: bn_stats/bn_aggr (VectorE) + sqrt (ScalarE) +
    reciprocal (VectorE), applied via one ScalarE activation with
    per-partition scale/bias.
  - The lone 49th tile runs as the FIRST group (small working set => early
    TensorE start) so the drain ends on a dense 4-tile group.
  - All matmuls are bf16 inputs with fp32 PSUM accumulation (~4e-3 L2 rel err).

Everything is compiled once per (CT, apply_gamma_beta) configuration and cached.
"""

import numpy as np
import ml_dtypes

P = 128
NODE_DIM = 512
EDGE_DIM = 96
HID = 1024
OUT = 512
N_NODES = 50000
N_EDGES = 800000
NCORES = 8
LN_EPS = 1e-5

NPC = N_NODES // NCORES          # 6250 nodes per core
T_TILES = -(-NPC // P)           # 49 node tiles per core
NPAD = T_TILES * P               # 6272
GMAX = 4                         # node tiles per super-tile (NT = 512 free dim)
D1 = 20                          # degree layers summed on the Vector engine

BF16 = ml_dtypes.bfloat16

_CACHE: dict = {}


# ----------------------------------------------------------------------------
# Bass program
# ----------------------------------------------------------------------------

def _build_program(ct: int, apply_gamma_beta: bool):
    import concourse.bass as bass
    import concourse.bacc as bacc
    import concourse.mybir as mybir
    import concourse.tile as tile

    f32 = mybir.dt.float32
    bf16 = mybir.dt.bfloat16
    Act = mybir.ActivationFunctionType
    Alu = mybir.AluOpType
    Ax = mybir.AxisListType

    KD = NODE_DIM // P           # 4 node-feat k-chunks
    KH = HID // P                # 8 hidden k-chunks
    MH = HID // P                # 8 hidden m-chunks
    KD1 = KD + 1                 # + 1 chunk for the 96 agg features
    D2 = D1 // 2                 # 10
    D4 = D1 // 4                 # 5

    nc = bacc.Bacc("TRN2", target_bir_lowering=False, debug=False)

    # inputs (per core)
    eflay_d = nc.declare_dram_parameter("eflay", [T_TILES, EDGE_DIM, P, D1], bf16, isOutput=False)
    eftl_d = nc.declare_dram_parameter("eftl", [P, T_TILES * ct * P], bf16, isOutput=False)
    dstl_d = nc.declare_dram_parameter("dstl", [P, T_TILES, ct], bf16, isOutput=False)
    # node features blocked [partition, tile, k, node-in-tile]: a group's DMA
    # is one 4KB-per-partition contiguous read (descriptor-efficient)
    nfg_d = nc.declare_dram_parameter("nfg", [P, T_TILES, KD, P], bf16, isOutput=False)
    w1_d = nc.declare_dram_parameter("w1", [P, KD1 * MH * P], bf16, isOutput=False)
    w2_d = nc.declare_dram_parameter("w2", [P, KH * MH * P], bf16, isOutput=False)
    w3_d = nc.declare_dram_parameter("w3", [P, KH * OUT], bf16, isOutput=False)
    # cstB: b1T(MH) | b2T(MH); cstLN: gamma(OUT) | beta(OUT) | b3(OUT) | eps(1)
    cstB_d = nc.declare_dram_parameter("cstB", [P, 2 * MH], f32, isOutput=False)
    cstLN_d = nc.declare_dram_parameter("cstLN", [P, 3 * OUT + 1], f32, isOutput=False)
    cstb_d = nc.declare_dram_parameter("cstb", [P, P], bf16, isOutput=False)
    y_d = nc.declare_dram_parameter("y", [NPAD, OUT], f32, isOutput=True)

    # tile 48 (the ragged one) runs first, then group sizes ramp 2, 3, 4...:
    # the start of the kernel is HBM-bound, so early groups keep their input
    # working set small enough to arrive in time.
    groups = [(T_TILES - 1, 1)]
    t0 = 0
    ramp = [2, 3]
    while t0 < T_TILES - 1:
        g = min(ramp.pop(0) if ramp else GMAX, T_TILES - 1 - t0)
        groups.append((t0, g))
        t0 += g

    with tile.TileContext(nc) as tc:
        with (
            tc.tile_pool(name="const", bufs=1) as constp,
            tc.tile_pool(name="lay", bufs=6) as layp,
            tc.tile_pool(name="lay2", bufs=4) as lay2p,
            tc.tile_pool(name="lay3", bufs=4) as lay3p,
            tc.tile_pool(name="ef", bufs=6) as efp,
            tc.tile_pool(name="oh", bufs=6) as ohp,
            tc.tile_pool(name="ag32", bufs=2) as ag32p,
            tc.tile_pool(name="agg", bufs=3) as aggp,
            tc.tile_pool(name="nfx", bufs=3) as nfxp,
            tc.tile_pool(name="h1", bufs=2) as h1p,
            tc.tile_pool(name="h2", bufs=2) as h2p,
            tc.tile_pool(name="yo", bufs=3) as yop,
            tc.tile_pool(name="st", bufs=8) as stp,
            tc.tile_pool(name="psA", bufs=2, space="PSUM") as psA,
            tc.tile_pool(name="psM", bufs=3, space="PSUM") as psM,
            tc.tile_pool(name="psY", bufs=3, space="PSUM") as psY,
        ):
            # small constants first so the tail path of group 0 clears quickly
            dstl_sb = constp.tile([P, T_TILES, ct], bf16)
            nc.sync.dma_start(out=dstl_sb[:], in_=dstl_d[:, :, :])
            cstb_sb = constp.tile([P, P], bf16)
            nc.sync.dma_start(out=cstb_sb[:], in_=cstb_d[:, :])
            cstB_sb = constp.tile([P, 2 * MH], f32)
            nc.sync.dma_start(out=cstB_sb[:], in_=cstB_d[:, :])
            # group 0's slice of the tail-edge table as its own tiny tile so
            # its one-hot matmuls never wait on the full-table load
            t00 = groups[0][0]
            eftl0_sb = constp.tile([P, ct * P], bf16)
            nc.sync.dma_start(out=eftl0_sb[:],
                              in_=eftl_d[:, t00 * ct * P:(t00 + 1) * ct * P])
            w1_sb = constp.tile([P, MH * KD1 * P], bf16)
            w2_sb = constp.tile([P, MH * KH * P], bf16)
            w3_sb = constp.tile([P, KH * OUT], bf16)
            cstLN_sb = constp.tile([P, 3 * OUT + 1], f32)
            # whole tail-edge table stays resident (12.5KB/partition); group
            # 1's slice loads at startup, the remainder during group 1's body
            eftl_sb = constp.tile([P, T_TILES * ct * P], bf16)
            t10, g1n = groups[1]
            nc.sync.dma_start(
                out=eftl_sb[:, t10 * ct * P:(t10 + g1n) * ct * P],
                in_=eftl_d[:, t10 * ct * P:(t10 + g1n) * ct * P])

            def emit_agg_dmas(tstart, g):
                """Issue the edge-data DMAs for a group one group ahead of
                their compute so the reduces never wait on them."""
                lays = []
                for s in range(g):
                    lay = layp.tile([EDGE_DIM, P, D1], bf16, tag="lay", name="lay")
                    nc.sync.dma_start(out=lay[:], in_=eflay_d[tstart + s, :, :, :])
                    lays.append(lay)
                return lays

            def alloc_agg():
                aggT = aggp.tile([EDGE_DIM, GMAX * P], bf16, tag="agg",
                                 name="aggT")
                agg32 = ag32p.tile([EDGE_DIM, GMAX * P], f32, tag="ag32",
                                   name="agg32")
                ps_t = psA.tile([P, GMAX * P], f32, tag="psA")
                return aggT, agg32, ps_t

            def emit_onehot(t):
                """One-hot of a tile's tail-edge destinations; emitted well
                ahead of the tail matmul so the PE FIFO never waits on it."""
                oh_t = ohp.tile([P, ct, P], bf16, tag="oh", name="oh_t")
                nc.vector.tensor_tensor(
                    out=oh_t[:],
                    in0=dstl_sb[:, t, :, None].to_broadcast([P, ct, P]),
                    in1=cstb_sb[:, None, 0:P].to_broadcast([P, ct, P]),
                    op=Alu.is_equal,
                )
                return oh_t

            def emit_agg_chain(t, s, lay, oh_t, aggT, agg32, ps_t, ef_src=None):
                """Segment-sum of node tile t into aggT[:, s*P:(s+1)*P].

                Vector engine does the bulk (degree-layered reduce); TensorE
                adds the high-degree tail via ct one-hot chunk matmuls into a
                PSUM bank shared by the whole group (no cross-subtile PSUM
                dependency)."""
                for j in range(ct):
                    if ef_src is None:
                        lhsT = eftl_sb[:, (t * ct + j) * P:(t * ct + j + 1) * P]
                    else:
                        lhsT = ef_src[:, j * P:(j + 1) * P]
                    nc.tensor.matmul(
                        out=ps_t[:, s * P:(s + 1) * P],
                        lhsT=lhsT,
                        rhs=oh_t[:, j, :],
                        start=(j == 0),
                        stop=(j == ct - 1),
                    )
                lay2 = lay2p.tile([EDGE_DIM, P, D2], bf16, tag="lay2")
                nc.vector.tensor_tensor(
                    out=lay2[:], in0=lay[:, :, 0:D2], in1=lay[:, :, D2:D1],
                    op=Alu.add,
                )
                lay3 = lay3p.tile([EDGE_DIM, P, D4], bf16, tag="lay3")
                nc.vector.tensor_tensor(
                    out=lay3[:], in0=lay2[:, :, 0:D4], in1=lay2[:, :, D4:D2],
                    op=Alu.add,
                )
                nc.vector.tensor_reduce(
                    out=agg32[:, s * P:(s + 1) * P],
                    in_=lay3[:], axis=Ax.X, op=Alu.add,
                )
                # combine tail + cast to bf16 for the L1 matmul rhs
                nc.vector.tensor_tensor(
                    out=aggT[:, s * P:(s + 1) * P],
                    in0=agg32[:, s * P:(s + 1) * P],
                    in1=ps_t[0:EDGE_DIM, s * P:(s + 1) * P],
                    op=Alu.add,
                )

            def emit_nfx_dma(tstart, g):
                nfx = nfxp.tile([P, GMAX, KD, P], bf16, tag="nfx")
                nc.sync.dma_start(out=nfx[:, 0:g, :, :],
                                  in_=nfg_d[:, tstart:tstart + g, :, :])
                return nfx

            # group 0's aggregation up front (weight DMAs stream in behind it)
            agg_tiles = {}
            agg_dmas = {}
            nfx_tiles = {}
            t00_, g00 = groups[0]
            # group 0's single layer tile is DMA'd in two node-halves so the
            # first reduce starts as soon as the first half lands
            lay_g0 = []
            for c0 in (0, P // 2):
                lh = layp.tile([EDGE_DIM, P // 2, D1], bf16, tag="lay",
                               name="lay")
                nc.sync.dma_start(out=lh[:],
                                  in_=eflay_d[t00_, :, c0:c0 + P // 2, :])
                lay_g0.append(lh)
            nfx_tiles[0] = emit_nfx_dma(*groups[0])
            nc.sync.dma_start(out=w1_sb[:, 0:KD1 * P], in_=w1_d[:, 0:KD1 * P])
            # L1 m=0's node-feature accumulation goes first in the PE queue:
            # it only needs nfx + the w1 m=0 slice, so TensorE starts several
            # us before the tail matmul's inputs land
            g0_ps_m0 = psM.tile([P, GMAX * P], f32, tag="psM")
            for k in range(KD):
                nc.tensor.matmul(
                    out=g0_ps_m0[:, 0:g00 * P],
                    lhsT=w1_sb[:, k * P:(k + 1) * P],
                    rhs=nfx_tiles[0][:, 0:g00, k, :],
                    start=(k == 0),
                    stop=False,
                )
            agg_tiles[0] = alloc_agg()
            aggT0, agg320, ps_t0 = agg_tiles[0]
            oh0 = emit_onehot(t00_)
            for j in range(ct):
                nc.tensor.matmul(
                    out=ps_t0[:, 0:P],
                    lhsT=eftl0_sb[:, j * P:(j + 1) * P],
                    rhs=oh0[:, j, :],
                    start=(j == 0),
                    stop=(j == ct - 1),
                )
            for lh, c0 in zip(lay_g0, (0, P // 2)):
                lay2 = lay2p.tile([EDGE_DIM, P // 2, D2], bf16, tag="lay2")
                nc.vector.tensor_tensor(
                    out=lay2[:], in0=lh[:, :, 0:D2], in1=lh[:, :, D2:D1],
                    op=Alu.add,
                )
                lay3 = lay3p.tile([EDGE_DIM, P // 2, D4], bf16, tag="lay3")
                nc.vector.tensor_tensor(
                    out=lay3[:], in0=lay2[:, :, 0:D4], in1=lay2[:, :, D4:D2],
                    op=Alu.add,
                )
                nc.vector.tensor_reduce(
                    out=agg320[:, c0:c0 + P // 2],
                    in_=lay3[:], axis=Ax.X, op=Alu.add,
                )
                nc.vector.tensor_tensor(
                    out=aggT0[:, c0:c0 + P // 2],
                    in0=agg320[:, c0:c0 + P // 2],
                    in1=ps_t0[0:EDGE_DIM, c0:c0 + P // 2],
                    op=Alu.add,
                )
            for m in range(1, MH):
                nc.sync.dma_start(
                    out=w1_sb[:, m * KD1 * P:(m + 1) * KD1 * P],
                    in_=w1_d[:, m * KD1 * P:(m + 1) * KD1 * P])

            for gi, (tstart, g) in enumerate(groups):
                nt = g * P  # free-dim width of this super-tile
                n0 = tstart * P
                aggT, _, _ = agg_tiles.pop(gi)
                nfx = nfx_tiles.pop(gi)

                # next group's input DMAs go out a full group early (except
                # behind group 0's weight DMAs: the ramp is HBM-bound and L1
                # needs w1 slices before any of group 1's edge data)
                if gi == 1:
                    # rest of the tail-edge table (tiles outside groups 0/1)
                    if t10 > 0:
                        nc.sync.dma_start(out=eftl_sb[:, 0:t10 * ct * P],
                                          in_=eftl_d[:, 0:t10 * ct * P])
                    if (t10 + g1n) < t00:
                        nc.sync.dma_start(
                            out=eftl_sb[:, (t10 + g1n) * ct * P:t00 * ct * P],
                            in_=eftl_d[:, (t10 + g1n) * ct * P:t00 * ct * P])
                if gi + 1 < len(groups) and gi > 0:
                    agg_dmas[gi + 1] = emit_agg_dmas(*groups[gi + 1])
                    nfx_tiles[gi + 1] = emit_nfx_dma(*groups[gi + 1])

                # ---- layer 1: h1T[m] = relu(W1.T @ xT + b1), x = [nf; agg] ----
                h1 = h1p.tile([P, KH, GMAX * P], bf16, tag="h1")
                for m in range(MH):
                    if gi == 0 and m == 0:
                        # node-feature chunks were pre-emitted before group 0's
                        # agg chains; only the agg chunk remains
                        ps = g0_ps_m0
                    else:
                        ps = psM.tile([P, GMAX * P], f32, tag="psM")
                        for k in range(KD):
                            nc.tensor.matmul(
                                out=ps[:, 0:nt],
                                lhsT=w1_sb[:, (m * KD1 + k) * P:(m * KD1 + k + 1) * P],
                                rhs=nfx[:, 0:g, k, :],
                                start=(k == 0),
                                stop=False,
                            )
                    nc.tensor.matmul(
                        out=ps[:, 0:nt],
                        lhsT=w1_sb[0:EDGE_DIM, (m * KD1 + KD) * P:(m * KD1 + KD) * P + P],
                        rhs=aggT[:, 0:nt],
                        start=False,
                        stop=True,
                    )
                    nc.scalar.activation(
                        out=h1[:, m, 0:nt], in_=ps[:, 0:nt], func=Act.Relu,
                        bias=cstB_sb[:, m:m + 1],
                    )
                    if gi == 0:
                        nc.sync.dma_start(
                            out=w2_sb[:, m * KH * P:(m + 1) * KH * P],
                            in_=w2_d[:, m * KH * P:(m + 1) * KH * P])
                        # interleave group 1's edge/node data between the w2
                        # slices: its reduces start during this group's L2,
                        # well before w2's tail is needed
                        if m == 1 and len(groups) > 1:
                            agg_dmas[1] = emit_agg_dmas(*groups[1])
                        if m == 3 and len(groups) > 1:
                            nfx_tiles[1] = emit_nfx_dma(*groups[1])

                # ---- layer 2 ----
                h2 = h2p.tile([P, KH, GMAX * P], bf16, tag="h2")
                for m in range(MH):
                    ps = psM.tile([P, GMAX * P], f32, tag="psM")
                    for k in range(KH):
                        nc.tensor.matmul(
                            out=ps[:, 0:nt],
                            lhsT=w2_sb[:, (m * KH + k) * P:(m * KH + k + 1) * P],
                            rhs=h1[:, k, 0:nt],
                            start=(k == 0),
                            stop=(k == KH - 1),
                        )
                    nc.scalar.activation(
                        out=h2[:, m, 0:nt], in_=ps[:, 0:nt], func=Act.Relu,
                        bias=cstB_sb[:, MH + m:MH + m + 1],
                    )
                    if gi == 0 and m < 2:
                        if m == 0:
                            nc.sync.dma_start(out=w3_sb[:], in_=w3_d[:, :])
                        else:
                            nc.sync.dma_start(out=cstLN_sb[:], in_=cstLN_d[:, :])

                # ---- layer 3 (nodes on partitions) + LayerNorm ----
                # the next group's aggregation interleaves per subtile, each
                # chain emitted AFTER that subtile's LN ops so the Vector FIFO
                # never delays the LN chain (psY slack absorbs the rest)
                if gi + 1 < len(groups):
                    tstart_nx, g_nx = groups[gi + 1]
                    agg_tiles[gi + 1] = alloc_agg()
                    lays_nx = agg_dmas.pop(gi + 1)
                    ohs_nx = [emit_onehot(tstart_nx + s) for s in range(g_nx)]
                else:
                    tstart_nx, g_nx = 0, 0
                for s in range(max(g, g_nx)):
                    if s >= g:
                        emit_agg_chain(tstart_nx + s, s, lays_nx[s],
                                       ohs_nx[s], *agg_tiles[gi + 1])
                        continue
                    ps_y = psY.tile([P, OUT], f32, tag="psY")
                    for k in range(KH):
                        nc.tensor.matmul(
                            out=ps_y[:],
                            lhsT=h2[:, k, s * P:(s + 1) * P],
                            rhs=w3_sb[:, k * OUT:(k + 1) * OUT],
                            start=(k == 0),
                            stop=(k == KH - 1),
                        )
                    # + b3 (broadcast rows) on VectorE, off the TensorE critical path
                    if gi == len(groups) - 1:
                        # drain-critical: halve the ops so the chain pipelines
                        HF = OUT // 2
                        st12 = stp.tile([P, 2, 6], f32, tag="st12")
                        for hh in range(2):
                            nc.vector.tensor_tensor(
                                out=ps_y[:, hh * HF:(hh + 1) * HF],
                                in0=ps_y[:, hh * HF:(hh + 1) * HF],
                                in1=cstLN_sb[:, 2 * OUT + hh * HF:
                                             2 * OUT + (hh + 1) * HF],
                                op=Alu.add,
                            )
                            nc.vector.bn_stats(st12[:, hh, :],
                                               ps_y[:, hh * HF:(hh + 1) * HF])
                        mv = stp.tile([P, 2], f32, tag="mv")
                        nc.vector.bn_aggr(mv[:], st12[:])
                    else:
                        nc.vector.tensor_tensor(
                            out=ps_y[:], in0=ps_y[:],
                            in1=cstLN_sb[:, 2 * OUT:3 * OUT],
                            op=Alu.add,
                        )
                        st6 = stp.tile([P, 6], f32, tag="st6")
                        nc.vector.bn_stats(st6[:], ps_y[:])
                        mv = stp.tile([P, 2], f32, tag="mv")
                        nc.vector.bn_aggr(mv[:], st6[:])
                    std = stp.tile([P, 1], f32, tag="std")
                    nc.scalar.activation(std[:], mv[:, 1:2], Act.Sqrt,
                                         bias=cstLN_sb[:, 3 * OUT:])
                    rstd = stp.tile([P, 1], f32, tag="rstd")
                    nc.vector.reciprocal(rstd[:], std[:])
                    nmr = stp.tile([P, 1], f32, tag="nmr")
                    nc.vector.tensor_scalar(
                        out=nmr[:], in0=mv[:, 0:1], scalar1=rstd[:], scalar2=-1.0,
                        op0=Alu.mult, op1=Alu.mult,
                    )
                    yn = yop.tile([P, OUT], f32, tag="yn")
                    r0 = (tstart + s) * P
                    if gi == len(groups) - 1 and not apply_gamma_beta:
                        HF = OUT // 2
                        for hh in range(2):
                            nc.scalar.activation(
                                out=yn[:, hh * HF:(hh + 1) * HF],
                                in_=ps_y[:, hh * HF:(hh + 1) * HF],
                                func=Act.Identity, bias=nmr[:], scale=rstd[:],
                            )
                            nc.sync.dma_start(
                                out=y_d[r0:r0 + P, hh * HF:(hh + 1) * HF],
                                in_=yn[:, hh * HF:(hh + 1) * HF])
                    else:
                        nc.scalar.activation(
                            out=yn[:], in_=ps_y[:], func=Act.Identity,
                            bias=nmr[:], scale=rstd[:],
                        )
                        if apply_gamma_beta:
                            nc.vector.tensor_tensor(
                                out=yn[:], in0=yn[:],
                                in1=cstLN_sb[:, 0:OUT], op=Alu.mult,
                            )
                            nc.vector.tensor_tensor(
                                out=yn[:], in0=yn[:],
                                in1=cstLN_sb[:, OUT:2 * OUT], op=Alu.add,
                            )
                        nc.sync.dma_start(out=y_d[r0:r0 + P, :], in_=yn[:])
                    if s < g_nx:
                        emit_agg_chain(tstart_nx + s, s, lays_nx[s],
                                       ohs_nx[s], *agg_tiles[gi + 1])

    nc.compile()
    return nc


# ----------------------------------------------------------------------------
# Host-side sharding / layout prep
# ----------------------------------------------------------------------------

def _prep_core(c, node_feat, edge_feat, dst, ct):
    KD_ = NODE_DIM // P
    lo = c * NPC
    sel = np.flatnonzero((dst >= lo) & (dst < lo + NPC))
    d = (dst[sel] - lo).astype(np.int64)
    order = np.argsort(d, kind="stable")
    sel = sel[order]
    d = d[order]
    counts = np.bincount(d, minlength=NPAD)
    offs = np.zeros(NPAD, np.int64)
    np.cumsum(counts[:-1], out=offs[1:])
    rank = np.arange(d.size) - offs[d]

    # main: first D1 edges of each node, degree-layered [T, 96, 128, D1]
    main = rank < D1
    flat = np.zeros((NPAD * D1, EDGE_DIM), np.float32)
    flat[d[main] * D1 + rank[main]] = edge_feat[sel[main]]
    eflay = np.ascontiguousarray(
        flat.astype(BF16).reshape(T_TILES, P, D1, EDGE_DIM).transpose(0, 3, 1, 2))

    # tail: edges beyond D1 per node, chunked one-hot layout per tile
    tail = np.flatnonzero(rank >= D1)
    dt_ = d[tail]
    tile_of = dt_ >> 7
    tcounts = np.bincount(tile_of, minlength=T_TILES)
    toffs = np.zeros(T_TILES, np.int64)
    np.cumsum(tcounts[:-1], out=toffs[1:])
    trank = np.arange(dt_.size) - toffs[tile_of]
    p_slot = trank % P
    c_slot = trank // P
    assert c_slot.max(initial=0) < ct

    eftl = np.zeros((T_TILES, P, ct, P), BF16)
    eftl[tile_of, p_slot, c_slot, :EDGE_DIM] = edge_feat[sel[tail]].astype(BF16)
    # resident layout: [partition(edge slot), tile*chunk*feat]
    eftl = np.ascontiguousarray(eftl.transpose(1, 0, 2, 3)).reshape(P, -1)
    dstl = np.full((T_TILES, P, ct), -1.0, BF16)
    dstl[tile_of, p_slot, c_slot] = (dt_ - (tile_of << 7)).astype(BF16)
    dstl = np.ascontiguousarray(dstl.transpose(1, 0, 2))

    # node features blocked [partition, tile, k, node-in-tile]:
    # nfg[p, t, k, j] = node_feat[t*128+j, k*128+p]
    nfp = np.zeros((NPAD, NODE_DIM), np.float32)
    nfp[:NPC] = node_feat[lo:lo + NPC]
    nfg = np.ascontiguousarray(
        nfp.astype(BF16).reshape(T_TILES, P, KD_, P).transpose(3, 0, 2, 1))
    return {"eflay": eflay, "eftl": eftl, "dstl": dstl, "nfg": nfg}


def _prep_shared(W1, b1, W2, b2, W3, b3, gamma, beta):
    KD1 = NODE_DIM // P + 1
    MH = HID // P
    KH = HID // P

    w1p = np.zeros((KD1 * P, HID), np.float32)
    w1p[:NODE_DIM + EDGE_DIM] = W1
    # m-major: col index (m*KD1 + k)*P + j
    w1 = np.ascontiguousarray(
        w1p.reshape(KD1, P, MH, P).transpose(1, 2, 0, 3)).reshape(P, -1).astype(BF16)
    w2 = np.ascontiguousarray(
        W2.reshape(KH, P, MH, P).transpose(1, 2, 0, 3)).reshape(P, -1).astype(BF16)
    w3 = np.ascontiguousarray(
        W3.reshape(KH, P, OUT).transpose(1, 0, 2)).reshape(P, -1).astype(BF16)

    cstB = np.ascontiguousarray(np.concatenate(
        [b1.reshape(MH, P).T, b2.reshape(MH, P).T], axis=1).astype(np.float32))
    cstLN = np.ascontiguousarray(np.concatenate([
        np.tile(gamma.reshape(1, OUT), (P, 1)),
        np.tile(beta.reshape(1, OUT), (P, 1)),
        np.tile(b3.reshape(1, OUT), (P, 1)),
        np.full((P, 1), LN_EPS, np.float32),
    ], axis=1).astype(np.float32))

    cstb = np.tile(np.arange(P, dtype=np.float32)[None, :], (P, 1)).astype(BF16)
    return {"w1": w1, "w2": w2, "w3": w3, "cstB": cstB, "cstLN": cstLN, "cstb": cstb}


# ----------------------------------------------------------------------------
# Entry point
# ----------------------------------------------------------------------------

def _ensure_axon_hooks_importable():
    """bass_utils imports antenv.axon_hooks when tracing is requested (even via
    the BASS_TRACE env var); provide a no-op stub if the module is absent so
    that path degrades to trace-skipped instead of crashing."""
    try:
        import antenv.axon_hooks  # noqa: F401
    except Exception:
        import sys
        import types
        try:
            import antenv
        except Exception:
            return
        mod = types.ModuleType('antenv.axon_hooks')
        mod._hook = None
        mod.set_axon_ntff_profile_hook = lambda h: setattr(mod, '_hook', h)
        mod.get_axon_ntff_profile_hook = lambda: mod._hook
        sys.modules['antenv.axon_hooks'] = mod
        antenv.axon_hooks = mod


def kernel(node_feat, edge_feat, edge_index, n_nodes, W1, b1, W2, b2, W3, b3,
           gamma, beta, _want_trace=False):
    from concourse.bass_utils import run_bass_kernel_spmd
    _ensure_axon_hooks_importable()

    node_feat = np.asarray(node_feat, dtype=np.float32)
    edge_feat = np.asarray(edge_feat, dtype=np.float32)
    edge_index = np.asarray(edge_index)
    assert int(n_nodes) == N_NODES
    assert node_feat.shape == (N_NODES, NODE_DIM)
    assert edge_feat.shape == (N_EDGES, EDGE_DIM)

    dst = edge_index[1].astype(np.int64)

    # tail capacity: chunks of 128 edges per tile beyond D1 per node (global,
    # so the SPMD program is shared by all cores)
    counts = np.bincount(dst, minlength=N_NODES)
    padded = np.zeros((NCORES, NPAD), np.int64)
    padded[:, :NPC] = counts.reshape(NCORES, NPC)
    tail_tile = np.maximum(padded - D1, 0).reshape(NCORES, T_TILES, P).sum(axis=2)
    ct = max(1, -(-int(tail_tile.max()) // P))

    gamma = np.asarray(gamma, dtype=np.float32)
    beta = np.asarray(beta, dtype=np.float32)
    apply_gb = not (np.all(gamma == 1.0) and np.all(beta == 0.0))

    key = (ct, apply_gb)
    if key not in _CACHE:
        _CACHE[key] = _build_program(ct, apply_gb)
    nc = _CACHE[key]

    shared = _prep_shared(
        np.asarray(W1, np.float32), np.asarray(b1, np.float32),
        np.asarray(W2, np.float32), np.asarray(b2, np.float32),
        np.asarray(W3, np.float32), np.asarray(b3, np.float32),
        gamma, beta)

    in_maps = []
    for c in range(NCORES):
        m = _prep_core(c, node_feat, edge_feat, dst, ct)
        m.update(shared)
        in_maps.append(m)

    res = run_bass_kernel_spmd(nc, in_maps, list(range(NCORES)), trace=_want_trace)

    y = np.concatenate([res.results[c]["y"][:NPC] for c in range(NCORES)], axis=0)
    out = np.ascontiguousarray(y, dtype=np.float32)
    if _want_trace:
        kernel.last_results = res
    return out


kernel.last_results = None
